# revision 1
# baseline (speedup 1.0000x reference)
"""GCN v2: ap_gather-based edge gather on 8 TRN2 cores.

Layout: nodes sharded 8 ways (core c owns dst range [cP,(c+1)P)).  Tables
live transposed in SBUF as [128 = 8 src-ranges x 16 feats, P nodes]; each
16-partition GPSIMD group gathers edges whose src falls in its range
(ap_gather, group-private int16 index lists).  Per (dst, range) segment
sums run on DVE (exact-K runs, K-desc order, SPMD-global structure);
partials are realigned to global dst order by a second ap_gather and
summed across ranges by one PE matmul.  Self-loop terms are added
directly from the core's own z'/h1' columns (no gather).  Both layers
share one index/schedule set since the graph is identical.
"""

import os
import sys

for _p in ("/opt/trn_rl_repo", "/opt/pypackages"):
    if _p not in sys.path:
        sys.path.insert(0, _p)

import numpy as np

from concourse import bacc, bass, tile, mybir, library_config
from concourse import bass_utils

F32 = mybir.dt.float32
I16 = mybir.dt.int16
AF = mybir.ActivationFunctionType
ALU = mybir.AluOpType

NC = 8
NI = 4096          # gather columns per ap_gather call

_last_result = {}


# ---------------------------------------------------------------------------
# Host-side plan
# ---------------------------------------------------------------------------

def _make_plan(src, dst, N):
    P = N // NC
    core_d = dst // P
    rng_s = src // P
    dloc = dst - core_d * P
    sloc = src - rng_s * P

    # per (core, range): dst counts
    K_cr = []        # K_cr[c][r] = dict-like arrays: (dsts_sorted, counts)
    for c in range(NC):
        row = []
        mc = core_d == c
        for r in range(NC):
            m = mc & (rng_s == r)
            d_ = dloc[m]
            s_ = sloc[m]
            cnt = np.bincount(d_, minlength=P)
            row.append((d_, s_, cnt))
        K_cr.append(row)

    # ONE K-structure shared by all (core, range) pairs so every reduce is
    # full-128-partition (DVE partition offsets must be multiples of 32).
    nd_g = {}
    for r in range(NC):
        for c in range(NC):
            cnt = K_cr[c][r][2]
            ks, nds = np.unique(cnt[cnt > 0], return_counts=True)
            for k, nd in zip(ks, nds):
                nd_g[int(k)] = max(nd_g.get(int(k), 0), int(nd))
    struct = [(k, nd_g[k]) for k in sorted(nd_g, reverse=True)]

    # chunked schedule: entries (coloff, K, nd, ppos); runs never straddle
    # chunk boundaries; identical for every class/core.
    sched = []
    ch = 0
    col = 0
    ppos = 1
    for (k, nd) in struct:
        left = nd
        while left > 0:
            while ch >= len(sched):
                sched.append([])
            fit = min(left, (NI - col) // k)
            if fit == 0:
                ch += 1
                col = 0
                continue
            sched[ch].append((col, k, fit, ppos))
            col += fit * k
            ppos += fit
            left -= fit
    NCH = len(sched)
    SL = NCH * NI
    PW = ppos + 2 - (ppos % 2)  # even pad
    # per-chunk used columns (mult of 16): trim the gather of the tail chunk
    used_cols = []
    for ch_e in sched:
        u = max(col + k * nd for (col, k, nd, _) in ch_e)
        used_cols.append(min(NI, ((u + 15) // 16) * 16))

    # per-core idx streams + partial position of each (dst, r)
    idx_data = np.zeros((NC, NCH, 128, NI // 16), dtype=np.int16)
    pos_cr = np.full((NC, NC, P), 0, dtype=np.int32)  # [c][r][dst] -> ppos
    for c in range(NC):
        for r in range(NC):
            d_, s_, cnt = K_cr[c][r]
            order = np.lexsort((s_, d_))
            d_s = d_[order]
            s_s = s_[order]
            starts = np.searchsorted(d_s, np.arange(P))
            ends = np.searchsorted(d_s, np.arange(P), side="right")
            # dsts grouped by K desc, dst asc
            ks = cnt.copy()
            # iterate global structure, fill real dsts
            by_k = {}
            for k in sorted(set(ks[ks > 0])):
                by_k[int(k)] = np.where(ks == k)[0]
            used = {int(k): 0 for k in by_k}
            stream = np.zeros(NCH * NI, dtype=np.int16)
            spos = 0  # global stream position (contiguous through chunks)
            # walk the same schedule the device uses
            for ch in range(NCH):
                base = ch * NI
                for (col, k, fit, ppos) in sched[ch]:
                    av = by_k.get(k, np.empty(0, np.int64))
                    u = used.get(k, 0)
                    take = av[u:u + fit]
                    used[k] = u + len(take)
                    for j, dd in enumerate(take):
                        sl = s_s[starts[dd]:ends[dd]]
                        stream[base + col + j * k: base + col + j * k + k] = sl
                        pos_cr[c, r, dd] = ppos + j
            # wrap into tiles: position i -> [16r + i%16, i//16]
            sw = stream.reshape(NCH, NI // 16, 16)
            idx_data[c, :, 16 * r:16 * r + 16, :] = sw.transpose(0, 2, 1)

    # realign indices: rid[c][r][j] = pos_cr or 0, j in [0, 12800)
    NDP = ((P + 511) // 512) * 512  # padded dst cols (512-mult)
    rid_data = np.zeros((NC, 128, NDP // 16), dtype=np.int16)
    for c in range(NC):
        for r in range(NC):
            v = np.zeros(NDP, dtype=np.int16)
            v[:P] = pos_cr[c, r].astype(np.int16)
            rid_data[c, 16 * r:16 * r + 16, :] = v.reshape(NDP // 16, 16).T
    return dict(P=P, SL=SL, NCH=NCH, PW=PW, NDP=NDP, sched=sched,
                used=used_cols, idx_data=idx_data, rid_data=rid_data)


# ---------------------------------------------------------------------------
# Device program
# ---------------------------------------------------------------------------

def _ap(t_ap, offset, dims):
    return bass.AP(t_ap.tensor, t_ap.offset + offset, [list(t_ap.ap[0])] + dims)


def _build(N, F, HID, C, plan):
    P = plan["P"]
    NCH = plan["NCH"]
    PW = plan["PW"]
    NDP = plan["NDP"]
    sched = plan["sched"]
    used = plan["used"]
    NT2 = NDP // 128          # logits tiles
    KC = F // 128

    nc = bacc.Bacc(None, target_bir_lowering=False, debug=False,
                   num_devices=NC)

    xT_d = nc.dram_tensor("xT", [F, NDP], F32, kind="ExternalInput")
    w1_d = nc.dram_tensor("W1", [F, HID], F32, kind="ExternalInput")
    b1_d = nc.dram_tensor("b1c", [16, 1], F32, kind="ExternalInput")
    w2_d = nc.dram_tensor("W2r", [HID, C], F32, kind="ExternalInput")
    b2_d = nc.dram_tensor("b2r", [128, C], F32, kind="ExternalInput")
    m16_d = nc.dram_tensor("M16", [128, HID], F32, kind="ExternalInput")
    dinv_d = nc.dram_tensor("dinv16", [16, NDP], F32, kind="ExternalInput")
    idx_d = nc.dram_tensor("idxs", [128, NCH * (NI // 16)], I16,
                           kind="ExternalInput")
    rid_d = nc.dram_tensor("rids", [128, NDP // 16], I16,
                           kind="ExternalInput")
    out_d = nc.dram_tensor("out", [128, NT2 * C], F32, kind="ExternalOutput")

    with tile.TileContext(nc) as tc:
        with (
            tc.tile_pool(name="const", bufs=1) as cp,
            tc.tile_pool(name="dram", bufs=1, space="DRAM") as dp,
            tc.tile_pool(name="xt", bufs=3) as xtp,
            tc.tile_pool(name="zp", bufs=2, space="PSUM") as zpp,
            tc.tile_pool(name="zs", bufs=2) as zsp,
            tc.tile_pool(name="tab", bufs=1) as tbp,
            tc.tile_pool(name="idx", bufs=1) as ixp,
            tc.tile_pool(name="g", bufs=2) as gp,
            tc.tile_pool(name="part", bufs=1) as pp,
            tc.tile_pool(name="ra", bufs=3) as rap,
            tc.tile_pool(name="post", bufs=3) as pop,
            tc.tile_pool(name="lp", bufs=2, space="PSUM") as lpp,
        ):
            nc.gpsimd.load_library(library_config.ap_gather)

            w1 = []
            for kc in range(KC):
                t = cp.tile([128, HID], F32, tag=f"w1_{kc}")
                nc.sync.dma_start(out=t[:],
                                  in_=w1_d[kc * 128:(kc + 1) * 128, :])
                w1.append(t)
            b1c = cp.tile([16, 1], F32, tag="b1c")
            nc.sync.dma_start(out=b1c[:], in_=b1_d[:, :])
            w2r = cp.tile([HID, C], F32, tag="w2r")
            nc.sync.dma_start(out=w2r[:], in_=w2_d[:, :])
            b2r = cp.tile([128, C], F32, tag="b2r")
            nc.sync.dma_start(out=b2r[:], in_=b2_d[:, :])
            m16 = cp.tile([128, HID], F32, tag="m16")
            nc.sync.dma_start(out=m16[:], in_=m16_d[:, :])
            idxs = cp.tile([128, NCH * (NI // 16)], I16, tag="idxs")
            nc.sync.dma_start(out=idxs[:], in_=idx_d[:, :])
            rids = cp.tile([128, NDP // 16], I16, tag="rids")
            nc.sync.dma_start(out=rids[:], in_=rid_d[:, :])

            zT_dram = dp.tile([16, NDP], F32, tag="zT")
            h1_dram = dp.tile([16, NDP], F32, tag="h1T")
            zAG_dram = dp.tile([16, P], F32, tag="zAG")
            h1AG_dram = dp.tile([16, P], F32, tag="h1AG")
            tb1_dram = dp.tile([128, P], F32, tag="tb1")
            tb2_dram = dp.tile([128, P], F32, tag="tb2")

            # ---- z'^T = dinv * (x @ W1)^T, in 512-col chunks ----
            for j in range(NDP // 512):
                zp = zpp.tile([16, 512], F32, tag="zp")
                for kc in range(KC):
                    xa = xtp.tile([128, 512], F32, tag="xa")
                    nc.sync.dma_start(
                        out=xa[:],
                        in_=xT_d[kc * 128:(kc + 1) * 128,
                                 j * 512:(j + 1) * 512])
                    nc.tensor.matmul(out=zp[:], lhsT=w1[kc][:], rhs=xa[:],
                                     start=(kc == 0), stop=(kc == KC - 1))
                dv = xtp.tile([16, 512], F32, tag="dv")
                nc.sync.dma_start(out=dv[:],
                                  in_=dinv_d[:, j * 512:(j + 1) * 512])
                zs = zsp.tile([16, 512], F32, tag="zs")
                nc.vector.tensor_tensor(out=zs[:], in0=zp[:], in1=dv[:],
                                        op=ALU.mult)
                nc.sync.dma_start(out=zT_dram[:, j * 512:(j + 1) * 512],
                                  in_=zs[:])

            nc.sync.dma_start(out=zAG_dram[:, :], in_=zT_dram[:, 0:P])
            nc.gpsimd.collective_compute(
                "AllGather", ALU.bypass,
                replica_groups=[list(range(NC))],
                ins=[zAG_dram[:, :]], outs=[tb1_dram[:, :]],
            )

            table = tbp.tile([128, P], F32, tag="table")
            partial = pp.tile([128, PW], F32, tag="partial")

            def layer(table_dram, self_dram, is_last):
                nc.sync.dma_start(out=table[:], in_=table_dram[:, :])
                nc.vector.memset(partial[:], 0.0)
                for ch in range(NCH):
                    u = used[ch]
                    gt = gp.tile([128, NI], F32, tag="gt")
                    nc.gpsimd.ap_gather(
                        out_ap=gt[:, 0:u], in_ap=table[:],
                        idxs_ap=idxs[:, ch * (NI // 16):
                                     ch * (NI // 16) + u // 16],
                        channels=128, num_elems=P, d=1, num_idxs=u,
                    )
                    for (col, k, nd, ppos) in sched[ch]:
                        nc.vector.tensor_reduce(
                            out=partial[:, ppos:ppos + nd],
                            in_=_ap(gt[:], col, [[k, nd], [1, k]]),
                            axis=mybir.AxisListType.X, op=ALU.add,
                        )
                # realign + combine + post, per 512-dst chunk
                RNI = 2048
                nrch = (NDP + RNI - 1) // RNI
                for rc in range(nrch):
                    w = min(RNI, NDP - rc * RNI)
                    ra = rap.tile([128, RNI], F32, tag="ra")
                    nc.gpsimd.ap_gather(
                        out_ap=ra[:, 0:w], in_ap=partial[:],
                        idxs_ap=rids[:, rc * (RNI // 16):
                                     rc * (RNI // 16) + w // 16],
                        channels=128, num_elems=PW, d=1, num_idxs=w,
                    )
                    for j in range(w // 512):
                        cols = slice(rc * RNI + j * 512,
                                     rc * RNI + j * 512 + 512)
                        ap_ = lpp.tile([16, 512], F32, tag="ap_")
                        nc.tensor.matmul(
                            out=ap_[:], lhsT=m16[:],
                            rhs=ra[:, j * 512:(j + 1) * 512],
                            start=True, stop=True)
                        sf = pop.tile([16, 512], F32, tag="sf")
                        nc.sync.dma_start(out=sf[:], in_=self_dram[:, cols])
                        dv = pop.tile([16, 512], F32, tag="dv2")
                        nc.sync.dma_start(out=dv[:], in_=dinv_d[:, cols])
                        ag = pop.tile([16, 512], F32, tag="ag")
                        nc.vector.tensor_tensor(out=ag[:], in0=ap_[:],
                                                in1=sf[:], op=ALU.add)
                        nc.vector.tensor_tensor(out=ag[:], in0=ag[:],
                                                in1=dv[:], op=ALU.mult)
                        if not is_last:
                            nc.vector.tensor_tensor(
                                out=ag[:], in0=ag[:],
                                in1=_ap(b1c[:], 0, [[0, 512]]), op=ALU.add)
                            nc.scalar.activation(out=ag[:], in_=ag[:],
                                                 func=AF.Relu)
                            nc.vector.tensor_tensor(out=ag[:], in0=ag[:],
                                                    in1=dv[:], op=ALU.mult)
                            nc.sync.dma_start(out=h1_dram[:, cols], in_=ag[:])
                        else:
                            # logits + log_softmax per 128-dst tile
                            for i in range(4):
                                t2 = (rc * RNI + j * 512) // 128 + i
                                lp = lpp.tile([128, C], F32, tag="lp")
                                nc.tensor.matmul(
                                    out=lp[:],
                                    lhsT=ag[:, i * 128:(i + 1) * 128],
                                    rhs=w2r[:], start=True, stop=True)
                                lt = pop.tile([128, C], F32, tag="lt")
                                nc.vector.tensor_tensor(
                                    out=lt[:], in0=lp[:], in1=b2r[:],
                                    op=ALU.add)
                                nm = pop.tile([128, 1], F32, tag="nm")
                                nc.vector.tensor_reduce(
                                    out=nm[:], in_=lt[:],
                                    axis=mybir.AxisListType.X,
                                    op=ALU.max, negate=True)
                                nc.vector.tensor_tensor(
                                    out=lt[:], in0=lt[:],
                                    in1=_ap(nm[:], 0, [[0, C]]), op=ALU.add)
                                et = pop.tile([128, C], F32, tag="et")
                                nc.scalar.activation(out=et[:], in_=lt[:],
                                                     func=AF.Exp)
                                nc.vector.tensor_reduce(
                                    out=nm[:], in_=et[:],
                                    axis=mybir.AxisListType.X, op=ALU.add)
                                nc.scalar.activation(out=nm[:], in_=nm[:],
                                                     func=AF.Ln)
                                nc.vector.tensor_tensor(
                                    out=lt[:], in0=lt[:],
                                    in1=_ap(nm[:], 0, [[0, C]]),
                                    op=ALU.subtract)
                                nc.sync.dma_start(
                                    out=out_d[:, t2 * C:(t2 + 1) * C],
                                    in_=lt[:])

            layer(tb1_dram, zT_dram, is_last=False)
            nc.sync.dma_start(out=h1AG_dram[:, :], in_=h1_dram[:, 0:P])
            nc.gpsimd.collective_compute(
                "AllGather", ALU.bypass,
                replica_groups=[list(range(NC))],
                ins=[h1AG_dram[:, :]], outs=[tb2_dram[:, :]],
            )
            layer(tb2_dram, h1_dram, is_last=True)

    return nc


# ---------------------------------------------------------------------------
# Entry point
# ---------------------------------------------------------------------------

def kernel(x, edge_index, W1, b1, W2, b2):
    N, F = x.shape
    HID = W1.shape[1]
    C = W2.shape[1]
    P = N // NC
    src = np.asarray(edge_index[0], dtype=np.int64)
    dst = np.asarray(edge_index[1], dtype=np.int64)

    deg = np.bincount(dst, minlength=N).astype(np.int64) + 1
    dinv = (1.0 / np.sqrt(deg.astype(np.float64))).astype(np.float32)

    plan = _make_plan(src, dst, N)
    NDP = plan["NDP"]
    nc = _build(N, F, HID, C, plan)

    x = np.asarray(x, dtype=np.float32)
    W1 = np.ascontiguousarray(np.asarray(W1, dtype=np.float32))
    W2 = np.ascontiguousarray(np.asarray(W2, dtype=np.float32))
    b2r = np.tile(np.asarray(b2, dtype=np.float32)[None, :], (128, 1))
    M16 = np.zeros((128, HID), dtype=np.float32)
    for r in range(NC):
        M16[16 * r:16 * r + 16, :] = np.eye(HID, dtype=np.float32)

    in_maps = []
    for c in range(NC):
        xT = np.zeros((F, NDP), dtype=np.float32)
        xT[:, :P] = x[c * P:(c + 1) * P].T
        d16 = np.zeros((16, NDP), dtype=np.float32)
        d16[:, :P] = dinv[c * P:(c + 1) * P][None, :]
        in_maps.append({
            "xT": np.ascontiguousarray(xT),
            "W1": W1,
            "b1c": np.ascontiguousarray(
                np.asarray(b1, np.float32).reshape(16, 1)),
            "W2r": W2,
            "b2r": b2r,
            "M16": M16,
            "dinv16": np.ascontiguousarray(d16),
            "idxs": np.ascontiguousarray(
                plan["idx_data"][c].transpose(1, 0, 2).reshape(128, -1)),
            "rids": np.ascontiguousarray(plan["rid_data"][c]),
        })

    trace = bool(int(os.environ.get("GCN_TRACE", "0")))
    if int(os.environ.get("GCN_SIM", "0")):
        from concourse.bass_interp import MultiCoreSim

        sim = MultiCoreSim(nc, num_cores=NC, trace=False)
        for c, core in enumerate(sim.cores.values()):
            for k, v in in_maps[c].items():
                core.tensor(k)[:] = v
        sim.simulate(check_with_hw=False)
        results = [{"out": np.array(core.tensor("out"))}
                   for core in sim.cores.values()]
        _last_result["exec_time_ns"] = None
    else:
        nc.finalize()
        br = bass_utils.run_bass_kernel_spmd(
            nc, in_maps, core_ids=list(range(NC)), trace=trace,
        )
        results = br.results
        _last_result["exec_time_ns"] = br.exec_time_ns

    _last_result["results"] = results
    _last_result["plan"] = plan

    out = np.empty((N, C), dtype=np.float32)
    for c in range(NC):
        arr = results[c]["out"].reshape(128, NDP // 128, C)
        arr = arr.transpose(1, 0, 2).reshape(NDP, C)
        out[c * P:(c + 1) * P] = arr[:P]
    return out



# revision 4
# speedup vs baseline: 1.0697x; 1.0697x over previous
"""GCN v2: ap_gather-based edge gather on 8 TRN2 cores.

Layout: nodes sharded 8 ways (core c owns dst range [cP,(c+1)P)).  Tables
live transposed in SBUF as [128 = 8 src-ranges x 16 feats, P nodes]; each
16-partition GPSIMD group gathers edges whose src falls in its range
(ap_gather, group-private int16 index lists).  Per (dst, range) segment
sums run on DVE (exact-K runs, K-desc order, SPMD-global structure);
partials are realigned to global dst order by a second ap_gather and
summed across ranges by one PE matmul.  Self-loop terms are added
directly from the core's own z'/h1' columns (no gather).  Both layers
share one index/schedule set since the graph is identical.
"""

import os
import sys

for _p in ("/opt/trn_rl_repo", "/opt/pypackages"):
    if _p not in sys.path:
        sys.path.insert(0, _p)

import numpy as np

from concourse import bacc, bass, tile, mybir, library_config
from concourse import bass_utils

F32 = mybir.dt.float32
I16 = mybir.dt.int16
AF = mybir.ActivationFunctionType
ALU = mybir.AluOpType

NC = 8
NI = 4096          # gather columns per ap_gather call

_last_result = {}


# ---------------------------------------------------------------------------
# Host-side plan
# ---------------------------------------------------------------------------

def _make_plan(src, dst, N):
    P = N // NC
    ZC = P  # zero column index (table has 16 zeroed pad columns at P..P+15)
    core_d = dst // P
    rng_s = src // P
    dloc = dst - core_d * P
    sloc = src - rng_s * P

    # per (core, range): dst counts
    K_cr = []        # K_cr[c][r] = dict-like arrays: (dsts_sorted, counts)
    KMAX = 0
    for c in range(NC):
        row = []
        mc = core_d == c
        for r in range(NC):
            m = mc & (rng_s == r)
            d_ = dloc[m]
            s_ = sloc[m]
            cnt = np.bincount(d_, minlength=P)
            KMAX = max(KMAX, int(cnt.max()))
            row.append((d_, s_, cnt))
        K_cr.append(row)

    # ONE slot-structure shared by all (core, range) pairs so every reduce is
    # full-128-partition.  A class-k dst may occupy a K'>=k slot, padding the
    # run with gathers of the zero column, so capacities only need to cover
    # the suffix maxima (near-zero padding) instead of per-class maxima.
    sufmax = np.zeros(KMAX + 2, dtype=np.int64)
    for c in range(NC):
        for r in range(NC):
            cnt = K_cr[c][r][2]
            ks, nds = np.unique(cnt[cnt > 0], return_counts=True)
            cc = np.zeros(KMAX + 2, dtype=np.int64)
            cc[ks] = nds
            suf = cc[::-1].cumsum()[::-1]
            np.maximum(sufmax, suf, out=sufmax)
    nd_g = sufmax - np.append(sufmax[1:], 0)
    struct = [(k, int(nd_g[k])) for k in range(KMAX, 0, -1) if nd_g[k] > 0]

    # chunked schedule: entries (coloff, K, nd, ppos); runs never straddle
    # chunk boundaries; identical for every class/core.
    sched = []
    ch = 0
    col = 0
    ppos = 1
    for (k, nd) in struct:
        left = nd
        while left > 0:
            while ch >= len(sched):
                sched.append([])
            fit = min(left, (NI - col) // k)
            if fit == 0:
                ch += 1
                col = 0
                continue
            sched[ch].append((col, k, fit, ppos))
            col += fit * k
            ppos += fit
            left -= fit
    NCH = len(sched)
    SL = NCH * NI
    PW = ppos + 2 - (ppos % 2)  # even pad
    # per-chunk used columns (mult of 16): trim the gather of the tail chunk
    used_cols = []
    for ch_e in sched:
        u = max(col + k * nd for (col, k, nd, _) in ch_e)
        used_cols.append(min(NI, ((u + 15) // 16) * 16))

    # per-core idx streams + partial position of each (dst, r)
    idx_data = np.full((NC, NCH, 128, NI // 16), ZC, dtype=np.int16)
    pos_cr = np.full((NC, NC, P), 0, dtype=np.int32)  # [c][r][dst] -> ppos
    for c in range(NC):
        for r in range(NC):
            d_, s_, cnt = K_cr[c][r]
            order = np.lexsort((s_, d_))
            d_s = d_[order]
            s_s = s_[order]
            starts = np.searchsorted(d_s, np.arange(P))
            ends = np.searchsorted(d_s, np.arange(P), side="right")
            # all dsts with count>0, sorted by count desc (dst asc within)
            nz = np.where(cnt > 0)[0]
            queue = nz[np.argsort(-cnt[nz], kind="stable")]
            qi = 0
            stream = np.full(NCH * NI, ZC, dtype=np.int16)
            # walk the same schedule the device uses (slots in K-desc order)
            for ch in range(NCH):
                base = ch * NI
                for (col, k, fit, ppos) in sched[ch]:
                    take = queue[qi:qi + fit]
                    qi += len(take)
                    for j, dd in enumerate(take):
                        kk = ends[dd] - starts[dd]
                        assert kk <= k, (kk, k)
                        sl = s_s[starts[dd]:ends[dd]]
                        o = base + col + j * k
                        stream[o:o + kk] = sl
                        pos_cr[c, r, dd] = ppos + j
            assert qi == len(queue), (qi, len(queue))
            # wrap into tiles: position i -> [16r + i%16, i//16]
            sw = stream.reshape(NCH, NI // 16, 16)
            idx_data[c, :, 16 * r:16 * r + 16, :] = sw.transpose(0, 2, 1)

    # realign indices: rid[c][r][j] = pos_cr or 0, j in [0, 12800)
    NDP = ((P + 511) // 512) * 512  # padded dst cols (512-mult)
    rid_data = np.zeros((NC, 128, NDP // 16), dtype=np.int16)
    for c in range(NC):
        for r in range(NC):
            v = np.zeros(NDP, dtype=np.int16)
            v[:P] = pos_cr[c, r].astype(np.int16)
            rid_data[c, 16 * r:16 * r + 16, :] = v.reshape(NDP // 16, 16).T
    return dict(P=P, SL=SL, NCH=NCH, PW=PW, NDP=NDP, sched=sched,
                used=used_cols, idx_data=idx_data, rid_data=rid_data)


# ---------------------------------------------------------------------------
# Device program
# ---------------------------------------------------------------------------

def _ap(t_ap, offset, dims):
    return bass.AP(t_ap.tensor, t_ap.offset + offset, [list(t_ap.ap[0])] + dims)


def _build(N, F, HID, C, plan):
    P = plan["P"]
    NCH = plan["NCH"]
    PW = plan["PW"]
    NDP = plan["NDP"]
    sched = plan["sched"]
    used = plan["used"]
    NT2 = NDP // 128          # logits tiles
    KC = F // 128

    nc = bacc.Bacc(None, target_bir_lowering=False, debug=False,
                   num_devices=NC)

    xT_d = nc.dram_tensor("xT", [F, NDP], F32, kind="ExternalInput")
    w1_d = nc.dram_tensor("W1", [F, HID], F32, kind="ExternalInput")
    b1_d = nc.dram_tensor("b1c", [16, 1], F32, kind="ExternalInput")
    w2_d = nc.dram_tensor("W2r", [HID, C], F32, kind="ExternalInput")
    b2_d = nc.dram_tensor("b2r", [128, C], F32, kind="ExternalInput")
    m16_d = nc.dram_tensor("M16", [128, HID], F32, kind="ExternalInput")
    dinv_d = nc.dram_tensor("dinv16", [16, NDP], F32, kind="ExternalInput")
    idx_d = nc.dram_tensor("idxs", [128, NCH * (NI // 16)], I16,
                           kind="ExternalInput")
    rid_d = nc.dram_tensor("rids", [128, NDP // 16], I16,
                           kind="ExternalInput")
    out_d = nc.dram_tensor("out", [128, NT2 * C], F32, kind="ExternalOutput")

    with tile.TileContext(nc) as tc:
        with (
            tc.tile_pool(name="const", bufs=1) as cp,
            tc.tile_pool(name="dram", bufs=1, space="DRAM") as dp,
            tc.tile_pool(name="xt", bufs=3) as xtp,
            tc.tile_pool(name="zp", bufs=2, space="PSUM") as zpp,
            tc.tile_pool(name="zs", bufs=2) as zsp,
            tc.tile_pool(name="tab", bufs=1) as tbp,
            tc.tile_pool(name="idx", bufs=1) as ixp,
            tc.tile_pool(name="g", bufs=2) as gp,
            tc.tile_pool(name="part", bufs=1) as pp,
            tc.tile_pool(name="ra", bufs=3) as rap,
            tc.tile_pool(name="post", bufs=3) as pop,
            tc.tile_pool(name="lp", bufs=2, space="PSUM") as lpp,
        ):
            nc.gpsimd.load_library(library_config.ap_gather)

            w1 = []
            for kc in range(KC):
                t = cp.tile([128, HID], F32, tag=f"w1_{kc}")
                nc.sync.dma_start(out=t[:],
                                  in_=w1_d[kc * 128:(kc + 1) * 128, :])
                w1.append(t)
            b1c = cp.tile([16, 1], F32, tag="b1c")
            nc.sync.dma_start(out=b1c[:], in_=b1_d[:, :])
            w2r = cp.tile([HID, C], F32, tag="w2r")
            nc.sync.dma_start(out=w2r[:], in_=w2_d[:, :])
            b2r = cp.tile([128, C], F32, tag="b2r")
            nc.sync.dma_start(out=b2r[:], in_=b2_d[:, :])
            m16 = cp.tile([128, HID], F32, tag="m16")
            nc.sync.dma_start(out=m16[:], in_=m16_d[:, :])
            idxs = cp.tile([128, NCH * (NI // 16)], I16, tag="idxs")
            nc.sync.dma_start(out=idxs[:], in_=idx_d[:, :])
            rids = cp.tile([128, NDP // 16], I16, tag="rids")
            nc.sync.dma_start(out=rids[:], in_=rid_d[:, :])

            zT_dram = dp.tile([16, NDP], F32, tag="zT")
            h1_dram = dp.tile([16, NDP], F32, tag="h1T")
            zAG_dram = dp.tile([16, P], F32, tag="zAG")
            h1AG_dram = dp.tile([16, P], F32, tag="h1AG")
            tb1_dram = dp.tile([128, P], F32, tag="tb1")
            tb2_dram = dp.tile([128, P], F32, tag="tb2")

            # ---- z'^T = dinv * (x @ W1)^T, in 512-col chunks ----
            for j in range(NDP // 512):
                zp = zpp.tile([16, 512], F32, tag="zp")
                for kc in range(KC):
                    xa = xtp.tile([128, 512], F32, tag="xa")
                    nc.sync.dma_start(
                        out=xa[:],
                        in_=xT_d[kc * 128:(kc + 1) * 128,
                                 j * 512:(j + 1) * 512])
                    nc.tensor.matmul(out=zp[:], lhsT=w1[kc][:], rhs=xa[:],
                                     start=(kc == 0), stop=(kc == KC - 1))
                dv = xtp.tile([16, 512], F32, tag="dv")
                nc.sync.dma_start(out=dv[:],
                                  in_=dinv_d[:, j * 512:(j + 1) * 512])
                zs = zsp.tile([16, 512], F32, tag="zs")
                nc.vector.tensor_tensor(out=zs[:], in0=zp[:], in1=dv[:],
                                        op=ALU.mult)
                nc.sync.dma_start(out=zT_dram[:, j * 512:(j + 1) * 512],
                                  in_=zs[:])

            nc.sync.dma_start(out=zAG_dram[:, :], in_=zT_dram[:, 0:P])
            nc.gpsimd.collective_compute(
                "AllGather", ALU.bypass,
                replica_groups=[list(range(NC))],
                ins=[zAG_dram[:, :]], outs=[tb1_dram[:, :]],
            )

            table = tbp.tile([128, P + 16], F32, tag="table")
            nc.vector.memset(table[:, P:P + 16], 0.0)
            partial = pp.tile([128, PW], F32, tag="partial")

            def layer(table_dram, self_dram, is_last):
                nc.sync.dma_start(out=table[:, 0:P], in_=table_dram[:, :])
                nc.vector.memset(partial[:], 0.0)
                for ch in range(NCH):
                    u = used[ch]
                    gt = gp.tile([128, NI], F32, tag="gt")
                    nc.gpsimd.ap_gather(
                        out_ap=gt[:, 0:u], in_ap=table[:],
                        idxs_ap=idxs[:, ch * (NI // 16):
                                     ch * (NI // 16) + u // 16],
                        channels=128, num_elems=P + 16, d=1, num_idxs=u,
                    )
                    for (col, k, nd, ppos) in sched[ch]:
                        nc.vector.tensor_reduce(
                            out=partial[:, ppos:ppos + nd],
                            in_=_ap(gt[:], col, [[k, nd], [1, k]]),
                            axis=mybir.AxisListType.X, op=ALU.add,
                        )
                # realign + combine + post, per 512-dst chunk
                RNI = 2048
                nrch = (NDP + RNI - 1) // RNI
                for rc in range(nrch):
                    w = min(RNI, NDP - rc * RNI)
                    ra = rap.tile([128, RNI], F32, tag="ra")
                    nc.gpsimd.ap_gather(
                        out_ap=ra[:, 0:w], in_ap=partial[:],
                        idxs_ap=rids[:, rc * (RNI // 16):
                                     rc * (RNI // 16) + w // 16],
                        channels=128, num_elems=PW, d=1, num_idxs=w,
                    )
                    for j in range(w // 512):
                        cols = slice(rc * RNI + j * 512,
                                     rc * RNI + j * 512 + 512)
                        ap_ = lpp.tile([16, 512], F32, tag="ap_")
                        nc.tensor.matmul(
                            out=ap_[:], lhsT=m16[:],
                            rhs=ra[:, j * 512:(j + 1) * 512],
                            start=True, stop=True)
                        sf = pop.tile([16, 512], F32, tag="sf")
                        nc.sync.dma_start(out=sf[:], in_=self_dram[:, cols])
                        dv = pop.tile([16, 512], F32, tag="dv2")
                        nc.sync.dma_start(out=dv[:], in_=dinv_d[:, cols])
                        ag = pop.tile([16, 512], F32, tag="ag")
                        nc.vector.tensor_tensor(out=ag[:], in0=ap_[:],
                                                in1=sf[:], op=ALU.add)
                        nc.vector.tensor_tensor(out=ag[:], in0=ag[:],
                                                in1=dv[:], op=ALU.mult)
                        if not is_last:
                            nc.vector.tensor_tensor(
                                out=ag[:], in0=ag[:],
                                in1=_ap(b1c[:], 0, [[0, 512]]), op=ALU.add)
                            nc.scalar.activation(out=ag[:], in_=ag[:],
                                                 func=AF.Relu)
                            nc.vector.tensor_tensor(out=ag[:], in0=ag[:],
                                                    in1=dv[:], op=ALU.mult)
                            nc.sync.dma_start(out=h1_dram[:, cols], in_=ag[:])
                        else:
                            # logits + log_softmax, 4x128 dsts batched
                            t2 = (rc * RNI + j * 512) // 128
                            lp = lpp.tile([128, 4 * C], F32, tag="lp")
                            for i in range(4):
                                nc.tensor.matmul(
                                    out=lp[:, i * C:(i + 1) * C],
                                    lhsT=ag[:, i * 128:(i + 1) * 128],
                                    rhs=w2r[:], start=True, stop=True)
                            lt = pop.tile([128, 4 * C], F32, tag="lt")
                            nc.vector.tensor_tensor(
                                out=lt[:], in0=lp[:],
                                in1=_ap(b2r[:], 0, [[0, 4], [1, C]]),
                                op=ALU.add)
                            nm = pop.tile([128, 4], F32, tag="nm")
                            nc.vector.tensor_reduce(
                                out=nm[:],
                                in_=_ap(lt[:], 0, [[C, 4], [1, C]]),
                                axis=mybir.AxisListType.X,
                                op=ALU.max, negate=True)
                            nc.vector.tensor_tensor(
                                out=lt[:], in0=lt[:],
                                in1=_ap(nm[:], 0, [[1, 4], [0, C]]),
                                op=ALU.add)
                            et = pop.tile([128, 4 * C], F32, tag="et")
                            nc.scalar.activation(out=et[:], in_=lt[:],
                                                 func=AF.Exp)
                            nc.vector.tensor_reduce(
                                out=nm[:],
                                in_=_ap(et[:], 0, [[C, 4], [1, C]]),
                                axis=mybir.AxisListType.X, op=ALU.add)
                            nc.scalar.activation(out=nm[:], in_=nm[:],
                                                 func=AF.Ln)
                            nc.vector.tensor_tensor(
                                out=lt[:], in0=lt[:],
                                in1=_ap(nm[:], 0, [[1, 4], [0, C]]),
                                op=ALU.subtract)
                            nc.sync.dma_start(
                                out=out_d[:, t2 * C:(t2 + 4) * C],
                                in_=lt[:])

            layer(tb1_dram, zT_dram, is_last=False)
            nc.sync.dma_start(out=h1AG_dram[:, :], in_=h1_dram[:, 0:P])
            nc.gpsimd.collective_compute(
                "AllGather", ALU.bypass,
                replica_groups=[list(range(NC))],
                ins=[h1AG_dram[:, :]], outs=[tb2_dram[:, :]],
            )
            layer(tb2_dram, h1_dram, is_last=True)

    return nc


# ---------------------------------------------------------------------------
# Entry point
# ---------------------------------------------------------------------------

def kernel(x, edge_index, W1, b1, W2, b2):
    N, F = x.shape
    HID = W1.shape[1]
    C = W2.shape[1]
    P = N // NC
    src = np.asarray(edge_index[0], dtype=np.int64)
    dst = np.asarray(edge_index[1], dtype=np.int64)

    deg = np.bincount(dst, minlength=N).astype(np.int64) + 1
    dinv = (1.0 / np.sqrt(deg.astype(np.float64))).astype(np.float32)

    plan = _make_plan(src, dst, N)
    NDP = plan["NDP"]
    nc = _build(N, F, HID, C, plan)

    x = np.asarray(x, dtype=np.float32)
    W1 = np.ascontiguousarray(np.asarray(W1, dtype=np.float32))
    W2 = np.ascontiguousarray(np.asarray(W2, dtype=np.float32))
    b2r = np.tile(np.asarray(b2, dtype=np.float32)[None, :], (128, 1))
    M16 = np.zeros((128, HID), dtype=np.float32)
    for r in range(NC):
        M16[16 * r:16 * r + 16, :] = np.eye(HID, dtype=np.float32)

    in_maps = []
    for c in range(NC):
        xT = np.zeros((F, NDP), dtype=np.float32)
        xT[:, :P] = x[c * P:(c + 1) * P].T
        d16 = np.zeros((16, NDP), dtype=np.float32)
        d16[:, :P] = dinv[c * P:(c + 1) * P][None, :]
        in_maps.append({
            "xT": np.ascontiguousarray(xT),
            "W1": W1,
            "b1c": np.ascontiguousarray(
                np.asarray(b1, np.float32).reshape(16, 1)),
            "W2r": W2,
            "b2r": b2r,
            "M16": M16,
            "dinv16": np.ascontiguousarray(d16),
            "idxs": np.ascontiguousarray(
                plan["idx_data"][c].transpose(1, 0, 2).reshape(128, -1)),
            "rids": np.ascontiguousarray(plan["rid_data"][c]),
        })

    trace = bool(int(os.environ.get("GCN_TRACE", "0")))
    if int(os.environ.get("GCN_SIM", "0")):
        from concourse.bass_interp import MultiCoreSim

        sim = MultiCoreSim(nc, num_cores=NC, trace=False)
        for c, core in enumerate(sim.cores.values()):
            for k, v in in_maps[c].items():
                core.tensor(k)[:] = v
        sim.simulate(check_with_hw=False)
        results = [{"out": np.array(core.tensor("out"))}
                   for core in sim.cores.values()]
        _last_result["exec_time_ns"] = None
    else:
        nc.finalize()
        br = bass_utils.run_bass_kernel_spmd(
            nc, in_maps, core_ids=list(range(NC)), trace=trace,
        )
        results = br.results
        _last_result["exec_time_ns"] = br.exec_time_ns

    _last_result["results"] = results
    _last_result["plan"] = plan

    out = np.empty((N, C), dtype=np.float32)
    for c in range(NC):
        arr = results[c]["out"].reshape(128, NDP // 128, C)
        arr = arr.transpose(1, 0, 2).reshape(NDP, C)
        out[c * P:(c + 1) * P] = arr[:P]
    return out



# revision 7
# speedup vs baseline: 1.0704x; 1.0007x over previous
"""GCN v2: ap_gather-based edge gather on 8 TRN2 cores.

Layout: nodes sharded 8 ways (core c owns dst range [cP,(c+1)P)).  Tables
live transposed in SBUF as [128 = 8 src-ranges x 16 feats, P nodes]; each
16-partition GPSIMD group gathers edges whose src falls in its range
(ap_gather, group-private int16 index lists).  Per (dst, range) segment
sums run on DVE (exact-K runs, K-desc order, SPMD-global structure);
partials are realigned to global dst order by a second ap_gather and
summed across ranges by one PE matmul.  Self-loop terms are added
directly from the core's own z'/h1' columns (no gather).  Both layers
share one index/schedule set since the graph is identical.
"""

import os
import sys

for _p in ("/opt/trn_rl_repo", "/opt/pypackages"):
    if _p not in sys.path:
        sys.path.insert(0, _p)

import numpy as np

from concourse import bacc, bass, tile, mybir, library_config
from concourse import bass_utils

F32 = mybir.dt.float32
I16 = mybir.dt.int16
AF = mybir.ActivationFunctionType
ALU = mybir.AluOpType

NC = 8
NI = 4096          # gather columns per ap_gather call

_last_result = {}


# ---------------------------------------------------------------------------
# Host-side plan
# ---------------------------------------------------------------------------

def _make_plan(src, dst, N):
    P = N // NC
    ZC = P  # zero column index (table has 16 zeroed pad columns at P..P+15)
    core_d = dst // P
    rng_s = src // P
    dloc = dst - core_d * P
    sloc = src - rng_s * P

    # per (core, range): dst counts
    K_cr = []        # K_cr[c][r] = dict-like arrays: (dsts_sorted, counts)
    KMAX = 0
    for c in range(NC):
        row = []
        mc = core_d == c
        for r in range(NC):
            m = mc & (rng_s == r)
            d_ = dloc[m]
            s_ = sloc[m]
            cnt = np.bincount(d_, minlength=P)
            KMAX = max(KMAX, int(cnt.max()))
            row.append((d_, s_, cnt))
        K_cr.append(row)

    # ONE slot-structure shared by all (core, range) pairs so every reduce is
    # full-128-partition.  A class-k dst may occupy a K'>=k slot, padding the
    # run with gathers of the zero column, so capacities only need to cover
    # the suffix maxima (near-zero padding) instead of per-class maxima.
    sufmax = np.zeros(KMAX + 2, dtype=np.int64)
    for c in range(NC):
        for r in range(NC):
            cnt = K_cr[c][r][2]
            ks, nds = np.unique(cnt[cnt > 0], return_counts=True)
            cc = np.zeros(KMAX + 2, dtype=np.int64)
            cc[ks] = nds
            suf = cc[::-1].cumsum()[::-1]
            np.maximum(sufmax, suf, out=sufmax)
    nd_g = sufmax - np.append(sufmax[1:], 0)
    struct = [(k, int(nd_g[k])) for k in range(KMAX, 0, -1) if nd_g[k] > 0]

    # chunked schedule: entries (coloff, K, nd, ppos); runs never straddle
    # chunk boundaries; identical for every class/core.
    sched = []
    ch = 0
    col = 0
    ppos = 1
    for (k, nd) in struct:
        left = nd
        while left > 0:
            while ch >= len(sched):
                sched.append([])
            fit = min(left, (NI - col) // k)
            if fit == 0:
                ch += 1
                col = 0
                continue
            sched[ch].append((col, k, fit, ppos))
            col += fit * k
            ppos += fit
            left -= fit
    NCH = len(sched)
    SL = NCH * NI
    PW = ppos + 2 - (ppos % 2)  # even pad
    # per-chunk used columns (mult of 16): trim the gather of the tail chunk
    used_cols = []
    for ch_e in sched:
        u = max(col + k * nd for (col, k, nd, _) in ch_e)
        used_cols.append(min(NI, ((u + 15) // 16) * 16))

    # per-core idx streams + partial position of each (dst, r)
    idx_data = np.full((NC, NCH, 128, NI // 16), ZC, dtype=np.int16)
    pos_cr = np.full((NC, NC, P), 0, dtype=np.int32)  # [c][r][dst] -> ppos
    for c in range(NC):
        for r in range(NC):
            d_, s_, cnt = K_cr[c][r]
            order = np.lexsort((s_, d_))
            d_s = d_[order]
            s_s = s_[order]
            starts = np.searchsorted(d_s, np.arange(P))
            ends = np.searchsorted(d_s, np.arange(P), side="right")
            # all dsts with count>0, sorted by count desc (dst asc within)
            nz = np.where(cnt > 0)[0]
            queue = nz[np.argsort(-cnt[nz], kind="stable")]
            qi = 0
            stream = np.full(NCH * NI, ZC, dtype=np.int16)
            # walk the same schedule the device uses (slots in K-desc order)
            for ch in range(NCH):
                base = ch * NI
                for (col, k, fit, ppos) in sched[ch]:
                    take = queue[qi:qi + fit]
                    qi += len(take)
                    for j, dd in enumerate(take):
                        kk = ends[dd] - starts[dd]
                        assert kk <= k, (kk, k)
                        sl = s_s[starts[dd]:ends[dd]]
                        o = base + col + j * k
                        stream[o:o + kk] = sl
                        pos_cr[c, r, dd] = ppos + j
            assert qi == len(queue), (qi, len(queue))
            # wrap into tiles: position i -> [16r + i%16, i//16]
            sw = stream.reshape(NCH, NI // 16, 16)
            idx_data[c, :, 16 * r:16 * r + 16, :] = sw.transpose(0, 2, 1)

    # realign indices: rid[c][r][j] = pos_cr or 0, j in [0, 12800)
    NDP = ((P + 511) // 512) * 512  # padded dst cols (512-mult)
    rid_data = np.zeros((NC, 128, NDP // 16), dtype=np.int16)
    for c in range(NC):
        for r in range(NC):
            v = np.zeros(NDP, dtype=np.int16)
            v[:P] = pos_cr[c, r].astype(np.int16)
            rid_data[c, 16 * r:16 * r + 16, :] = v.reshape(NDP // 16, 16).T
    return dict(P=P, SL=SL, NCH=NCH, PW=PW, NDP=NDP, sched=sched,
                used=used_cols, idx_data=idx_data, rid_data=rid_data)


# ---------------------------------------------------------------------------
# Device program
# ---------------------------------------------------------------------------

def _ap(t_ap, offset, dims):
    return bass.AP(t_ap.tensor, t_ap.offset + offset, [list(t_ap.ap[0])] + dims)


def _build(N, F, HID, C, plan):
    P = plan["P"]
    NCH = plan["NCH"]
    PW = plan["PW"]
    NDP = plan["NDP"]
    sched = plan["sched"]
    used = plan["used"]
    NT2 = NDP // 128          # logits tiles
    KC = F // 128

    nc = bacc.Bacc(None, target_bir_lowering=False, debug=False,
                   num_devices=NC)

    xT_d = nc.dram_tensor("xT", [F, NDP], F32, kind="ExternalInput")
    w1_d = nc.dram_tensor("W1", [F, HID], F32, kind="ExternalInput")
    b1_d = nc.dram_tensor("b1c", [16, 1], F32, kind="ExternalInput")
    w2_d = nc.dram_tensor("W2r", [HID, C], F32, kind="ExternalInput")
    b2_d = nc.dram_tensor("b2r", [128, C], F32, kind="ExternalInput")
    m16_d = nc.dram_tensor("M16", [128, HID], F32, kind="ExternalInput")
    dinv_d = nc.dram_tensor("dinv16", [16, NDP], F32, kind="ExternalInput")
    idx_d = nc.dram_tensor("idxs", [128, NCH * (NI // 16)], I16,
                           kind="ExternalInput")
    rid_d = nc.dram_tensor("rids", [128, NDP // 16], I16,
                           kind="ExternalInput")
    out_d = nc.dram_tensor("out", [128, NT2 * C], F32, kind="ExternalOutput")

    with tile.TileContext(nc) as tc:
        with (
            tc.tile_pool(name="const", bufs=1) as cp,
            tc.tile_pool(name="dram", bufs=1, space="DRAM") as dp,
            tc.tile_pool(name="xt", bufs=3) as xtp,
            tc.tile_pool(name="zp", bufs=2, space="PSUM") as zpp,
            tc.tile_pool(name="zs", bufs=2) as zsp,
            tc.tile_pool(name="tab", bufs=1) as tbp,
            tc.tile_pool(name="idx", bufs=1) as ixp,
            tc.tile_pool(name="g", bufs=2) as gp,
            tc.tile_pool(name="part", bufs=1) as pp,
            tc.tile_pool(name="ra", bufs=3) as rap,
            tc.tile_pool(name="post", bufs=3) as pop,
            tc.tile_pool(name="lp", bufs=2, space="PSUM") as lpp,
        ):
            nc.gpsimd.load_library(library_config.ap_gather)

            w1 = []
            for kc in range(KC):
                t = cp.tile([128, HID], F32, tag=f"w1_{kc}")
                nc.sync.dma_start(out=t[:],
                                  in_=w1_d[kc * 128:(kc + 1) * 128, :])
                w1.append(t)
            b1c = cp.tile([16, 1], F32, tag="b1c")
            nc.sync.dma_start(out=b1c[:], in_=b1_d[:, :])
            w2r = cp.tile([HID, C], F32, tag="w2r")
            nc.sync.dma_start(out=w2r[:], in_=w2_d[:, :])
            b2r = cp.tile([128, C], F32, tag="b2r")
            nc.sync.dma_start(out=b2r[:], in_=b2_d[:, :])
            m16 = cp.tile([128, HID], F32, tag="m16")
            nc.sync.dma_start(out=m16[:], in_=m16_d[:, :])
            idxs = cp.tile([128, NCH * (NI // 16)], I16, tag="idxs")
            nc.sync.dma_start(out=idxs[:], in_=idx_d[:, :])
            rids = cp.tile([128, NDP // 16], I16, tag="rids")
            nc.sync.dma_start(out=rids[:], in_=rid_d[:, :])

            zT_dram = dp.tile([16, NDP], F32, tag="zT")
            h1_dram = dp.tile([16, NDP], F32, tag="h1T")
            zAG_dram = dp.tile([16, P], F32, tag="zAG")
            h1AG_dram = dp.tile([16, P], F32, tag="h1AG")
            tb1_dram = dp.tile([128, P], F32, tag="tb1")
            tb2_dram = dp.tile([128, P], F32, tag="tb2")

            # ---- z'^T = dinv * (x @ W1)^T, in 512-col chunks ----
            dma_engines = [nc.sync, nc.scalar]
            for j in range(NDP // 512):
                zp = zpp.tile([16, 512], F32, tag="zp")
                for kc in range(KC):
                    xa = xtp.tile([128, 512], F32, tag="xa")
                    dma_engines[(j * KC + kc) % 2].dma_start(
                        out=xa[:],
                        in_=xT_d[kc * 128:(kc + 1) * 128,
                                 j * 512:(j + 1) * 512])
                    nc.tensor.matmul(out=zp[:], lhsT=w1[kc][:], rhs=xa[:],
                                     start=(kc == 0), stop=(kc == KC - 1))
                dv = xtp.tile([16, 512], F32, tag="dv")
                nc.sync.dma_start(out=dv[:],
                                  in_=dinv_d[:, j * 512:(j + 1) * 512])
                zs = zsp.tile([16, 512], F32, tag="zs")
                nc.vector.tensor_tensor(out=zs[:], in0=zp[:], in1=dv[:],
                                        op=ALU.mult)
                nc.sync.dma_start(out=zT_dram[:, j * 512:(j + 1) * 512],
                                  in_=zs[:])

            nc.sync.dma_start(out=zAG_dram[:, :], in_=zT_dram[:, 0:P])
            nc.gpsimd.collective_compute(
                "AllGather", ALU.bypass,
                replica_groups=[list(range(NC))],
                ins=[zAG_dram[:, :]], outs=[tb1_dram[:, :]],
            )

            table = tbp.tile([128, P + 16], F32, tag="table")
            nc.vector.memset(table[:, P:P + 16], 0.0)
            partial = pp.tile([128, PW], F32, tag="partial")

            def layer(table_dram, self_dram, is_last):
                # spread the 6.4MB table load across engine DGE queues
                q = P // 4
                nc.sync.dma_start(out=table[:, 0:q],
                                  in_=table_dram[:, 0:q])
                nc.scalar.dma_start(out=table[:, q:2 * q],
                                    in_=table_dram[:, q:2 * q])
                nc.gpsimd.dma_start(out=table[:, 2 * q:3 * q],
                                    in_=table_dram[:, 2 * q:3 * q])
                nc.sync.dma_start(out=table[:, 3 * q:P],
                                  in_=table_dram[:, 3 * q:P])
                nc.vector.memset(partial[:], 0.0)
                for ch in range(NCH):
                    u = used[ch]
                    gt = gp.tile([128, NI], F32, tag="gt")
                    nc.gpsimd.ap_gather(
                        out_ap=gt[:, 0:u], in_ap=table[:],
                        idxs_ap=idxs[:, ch * (NI // 16):
                                     ch * (NI // 16) + u // 16],
                        channels=128, num_elems=P + 16, d=1, num_idxs=u,
                    )
                    for (col, k, nd, ppos) in sched[ch]:
                        nc.vector.tensor_reduce(
                            out=partial[:, ppos:ppos + nd],
                            in_=_ap(gt[:], col, [[k, nd], [1, k]]),
                            axis=mybir.AxisListType.X, op=ALU.add,
                        )
                # realign + combine + post, per 512-dst chunk
                RNI = 2048
                nrch = (NDP + RNI - 1) // RNI
                for rc in range(nrch):
                    w = min(RNI, NDP - rc * RNI)
                    ra = rap.tile([128, RNI], F32, tag="ra")
                    nc.gpsimd.ap_gather(
                        out_ap=ra[:, 0:w], in_ap=partial[:],
                        idxs_ap=rids[:, rc * (RNI // 16):
                                     rc * (RNI // 16) + w // 16],
                        channels=128, num_elems=PW, d=1, num_idxs=w,
                    )
                    for j in range(w // 512):
                        cols = slice(rc * RNI + j * 512,
                                     rc * RNI + j * 512 + 512)
                        ap_ = lpp.tile([16, 512], F32, tag="ap_")
                        nc.tensor.matmul(
                            out=ap_[:], lhsT=m16[:],
                            rhs=ra[:, j * 512:(j + 1) * 512],
                            start=True, stop=True)
                        sf = pop.tile([16, 512], F32, tag="sf")
                        nc.sync.dma_start(out=sf[:], in_=self_dram[:, cols])
                        dv = pop.tile([16, 512], F32, tag="dv2")
                        nc.sync.dma_start(out=dv[:], in_=dinv_d[:, cols])
                        ag = pop.tile([16, 512], F32, tag="ag")
                        nc.vector.tensor_tensor(out=ag[:], in0=ap_[:],
                                                in1=sf[:], op=ALU.add)
                        nc.vector.tensor_tensor(out=ag[:], in0=ag[:],
                                                in1=dv[:], op=ALU.mult)
                        if not is_last:
                            nc.vector.tensor_tensor(
                                out=ag[:], in0=ag[:],
                                in1=_ap(b1c[:], 0, [[0, 512]]), op=ALU.add)
                            nc.scalar.activation(out=ag[:], in_=ag[:],
                                                 func=AF.Relu)
                            nc.vector.tensor_tensor(out=ag[:], in0=ag[:],
                                                    in1=dv[:], op=ALU.mult)
                            nc.sync.dma_start(out=h1_dram[:, cols], in_=ag[:])
                        else:
                            # logits + log_softmax, 4x128 dsts batched
                            t2 = (rc * RNI + j * 512) // 128
                            lp = lpp.tile([128, 4 * C], F32, tag="lp")
                            for i in range(4):
                                nc.tensor.matmul(
                                    out=lp[:, i * C:(i + 1) * C],
                                    lhsT=ag[:, i * 128:(i + 1) * 128],
                                    rhs=w2r[:], start=True, stop=True)
                            lt = pop.tile([128, 4 * C], F32, tag="lt")
                            nc.vector.tensor_tensor(
                                out=lt[:], in0=lp[:],
                                in1=_ap(b2r[:], 0, [[0, 4], [1, C]]),
                                op=ALU.add)
                            nm = pop.tile([128, 4], F32, tag="nm")
                            nc.vector.tensor_reduce(
                                out=nm[:],
                                in_=_ap(lt[:], 0, [[C, 4], [1, C]]),
                                axis=mybir.AxisListType.X,
                                op=ALU.max, negate=True)
                            nc.vector.tensor_tensor(
                                out=lt[:], in0=lt[:],
                                in1=_ap(nm[:], 0, [[1, 4], [0, C]]),
                                op=ALU.add)
                            et = pop.tile([128, 4 * C], F32, tag="et")
                            nc.scalar.activation(out=et[:], in_=lt[:],
                                                 func=AF.Exp)
                            nc.vector.tensor_reduce(
                                out=nm[:],
                                in_=_ap(et[:], 0, [[C, 4], [1, C]]),
                                axis=mybir.AxisListType.X, op=ALU.add)
                            nc.scalar.activation(out=nm[:], in_=nm[:],
                                                 func=AF.Ln)
                            nc.vector.tensor_tensor(
                                out=lt[:], in0=lt[:],
                                in1=_ap(nm[:], 0, [[1, 4], [0, C]]),
                                op=ALU.subtract)
                            nc.sync.dma_start(
                                out=out_d[:, t2 * C:(t2 + 4) * C],
                                in_=lt[:])

            layer(tb1_dram, zT_dram, is_last=False)
            nc.sync.dma_start(out=h1AG_dram[:, :], in_=h1_dram[:, 0:P])
            nc.gpsimd.collective_compute(
                "AllGather", ALU.bypass,
                replica_groups=[list(range(NC))],
                ins=[h1AG_dram[:, :]], outs=[tb2_dram[:, :]],
            )
            layer(tb2_dram, h1_dram, is_last=True)

    return nc


# ---------------------------------------------------------------------------
# Entry point
# ---------------------------------------------------------------------------

def kernel(x, edge_index, W1, b1, W2, b2):
    N, F = x.shape
    HID = W1.shape[1]
    C = W2.shape[1]
    P = N // NC
    src = np.asarray(edge_index[0], dtype=np.int64)
    dst = np.asarray(edge_index[1], dtype=np.int64)

    deg = np.bincount(dst, minlength=N).astype(np.int64) + 1
    dinv = (1.0 / np.sqrt(deg.astype(np.float64))).astype(np.float32)

    plan = _make_plan(src, dst, N)
    NDP = plan["NDP"]
    nc = _build(N, F, HID, C, plan)

    x = np.asarray(x, dtype=np.float32)
    W1 = np.ascontiguousarray(np.asarray(W1, dtype=np.float32))
    W2 = np.ascontiguousarray(np.asarray(W2, dtype=np.float32))
    b2r = np.tile(np.asarray(b2, dtype=np.float32)[None, :], (128, 1))
    M16 = np.zeros((128, HID), dtype=np.float32)
    for r in range(NC):
        M16[16 * r:16 * r + 16, :] = np.eye(HID, dtype=np.float32)

    in_maps = []
    for c in range(NC):
        xT = np.zeros((F, NDP), dtype=np.float32)
        xT[:, :P] = x[c * P:(c + 1) * P].T
        d16 = np.zeros((16, NDP), dtype=np.float32)
        d16[:, :P] = dinv[c * P:(c + 1) * P][None, :]
        in_maps.append({
            "xT": np.ascontiguousarray(xT),
            "W1": W1,
            "b1c": np.ascontiguousarray(
                np.asarray(b1, np.float32).reshape(16, 1)),
            "W2r": W2,
            "b2r": b2r,
            "M16": M16,
            "dinv16": np.ascontiguousarray(d16),
            "idxs": np.ascontiguousarray(
                plan["idx_data"][c].transpose(1, 0, 2).reshape(128, -1)),
            "rids": np.ascontiguousarray(plan["rid_data"][c]),
        })

    trace = bool(int(os.environ.get("GCN_TRACE", "0")))
    if int(os.environ.get("GCN_SIM", "0")):
        from concourse.bass_interp import MultiCoreSim

        sim = MultiCoreSim(nc, num_cores=NC, trace=False)
        for c, core in enumerate(sim.cores.values()):
            for k, v in in_maps[c].items():
                core.tensor(k)[:] = v
        sim.simulate(check_with_hw=False)
        results = [{"out": np.array(core.tensor("out"))}
                   for core in sim.cores.values()]
        _last_result["exec_time_ns"] = None
    else:
        nc.finalize()
        br = bass_utils.run_bass_kernel_spmd(
            nc, in_maps, core_ids=list(range(NC)), trace=trace,
        )
        results = br.results
        _last_result["exec_time_ns"] = br.exec_time_ns

    _last_result["results"] = results
    _last_result["plan"] = plan

    out = np.empty((N, C), dtype=np.float32)
    for c in range(NC):
        arr = results[c]["out"].reshape(128, NDP // 128, C)
        arr = arr.transpose(1, 0, 2).reshape(NDP, C)
        out[c * P:(c + 1) * P] = arr[:P]
    return out



# revision 9
# speedup vs baseline: 1.0750x; 1.0043x over previous
"""GCN v2: ap_gather-based edge gather on 8 TRN2 cores.

Layout: nodes sharded 8 ways (core c owns dst range [cP,(c+1)P)).  Tables
live transposed in SBUF as [128 = 8 src-ranges x 16 feats, P nodes]; each
16-partition GPSIMD group gathers edges whose src falls in its range
(ap_gather, group-private int16 index lists).  Per (dst, range) segment
sums run on DVE (exact-K runs, K-desc order, SPMD-global structure);
partials are realigned to global dst order by a second ap_gather and
summed across ranges by one PE matmul.  Self-loop terms are added
directly from the core's own z'/h1' columns (no gather).  Both layers
share one index/schedule set since the graph is identical.
"""

import os
import sys

for _p in ("/opt/trn_rl_repo", "/opt/pypackages"):
    if _p not in sys.path:
        sys.path.insert(0, _p)

import numpy as np

from concourse import bacc, bass, tile, mybir, library_config
from concourse import bass_utils

F32 = mybir.dt.float32
I16 = mybir.dt.int16
AF = mybir.ActivationFunctionType
ALU = mybir.AluOpType

NC = 8
NI = 4096          # gather columns per ap_gather call

_last_result = {}


# ---------------------------------------------------------------------------
# Host-side plan
# ---------------------------------------------------------------------------

def _make_plan(src, dst, N):
    P = N // NC
    ZC = P  # zero column index (table has 16 zeroed pad columns at P..P+15)
    core_d = dst // P
    rng_s = src // P
    dloc = dst - core_d * P
    sloc = src - rng_s * P

    # per (core, range): dst counts
    K_cr = []        # K_cr[c][r] = dict-like arrays: (dsts_sorted, counts)
    KMAX = 0
    for c in range(NC):
        row = []
        mc = core_d == c
        for r in range(NC):
            m = mc & (rng_s == r)
            d_ = dloc[m]
            s_ = sloc[m]
            cnt = np.bincount(d_, minlength=P)
            KMAX = max(KMAX, int(cnt.max()))
            row.append((d_, s_, cnt))
        K_cr.append(row)

    # ONE slot-structure shared by all (core, range) pairs so every reduce is
    # full-128-partition.  A class-k dst may occupy a K'>=k slot, padding the
    # run with gathers of the zero column, so capacities only need to cover
    # the suffix maxima (near-zero padding) instead of per-class maxima.
    sufmax = np.zeros(KMAX + 2, dtype=np.int64)
    for c in range(NC):
        for r in range(NC):
            cnt = K_cr[c][r][2]
            ks, nds = np.unique(cnt[cnt > 0], return_counts=True)
            cc = np.zeros(KMAX + 2, dtype=np.int64)
            cc[ks] = nds
            suf = cc[::-1].cumsum()[::-1]
            np.maximum(sufmax, suf, out=sufmax)
    nd_g = sufmax - np.append(sufmax[1:], 0)
    struct = [(k, int(nd_g[k])) for k in range(KMAX, 0, -1) if nd_g[k] > 0]

    # chunked schedule: entries (coloff, K, nd, ppos); runs never straddle
    # chunk boundaries; identical for every class/core.
    sched = []
    ch = 0
    col = 0
    ppos = 1
    for (k, nd) in struct:
        left = nd
        while left > 0:
            while ch >= len(sched):
                sched.append([])
            fit = min(left, (NI - col) // k)
            if fit == 0:
                ch += 1
                col = 0
                continue
            sched[ch].append((col, k, fit, ppos))
            col += fit * k
            ppos += fit
            left -= fit
    NCH = len(sched)
    SL = NCH * NI
    PW = ppos + 2 - (ppos % 2)  # even pad
    # per-chunk used columns (mult of 16): trim the gather of the tail chunk
    used_cols = []
    for ch_e in sched:
        u = max(col + k * nd for (col, k, nd, _) in ch_e)
        used_cols.append(min(NI, ((u + 15) // 16) * 16))

    # per-core idx streams + partial position of each (dst, r)
    idx_data = np.full((NC, NCH, 128, NI // 16), ZC, dtype=np.int16)
    pos_cr = np.full((NC, NC, P), 0, dtype=np.int32)  # [c][r][dst] -> ppos
    for c in range(NC):
        for r in range(NC):
            d_, s_, cnt = K_cr[c][r]
            order = np.lexsort((s_, d_))
            d_s = d_[order]
            s_s = s_[order]
            starts = np.searchsorted(d_s, np.arange(P))
            ends = np.searchsorted(d_s, np.arange(P), side="right")
            # all dsts with count>0, sorted by count desc (dst asc within)
            nz = np.where(cnt > 0)[0]
            queue = nz[np.argsort(-cnt[nz], kind="stable")]
            qi = 0
            stream = np.full(NCH * NI, ZC, dtype=np.int16)
            # walk the same schedule the device uses (slots in K-desc order)
            for ch in range(NCH):
                base = ch * NI
                for (col, k, fit, ppos) in sched[ch]:
                    take = queue[qi:qi + fit]
                    qi += len(take)
                    for j, dd in enumerate(take):
                        kk = ends[dd] - starts[dd]
                        assert kk <= k, (kk, k)
                        sl = s_s[starts[dd]:ends[dd]]
                        o = base + col + j * k
                        stream[o:o + kk] = sl
                        pos_cr[c, r, dd] = ppos + j
            assert qi == len(queue), (qi, len(queue))
            # wrap into tiles: position i -> [16r + i%16, i//16]
            sw = stream.reshape(NCH, NI // 16, 16)
            idx_data[c, :, 16 * r:16 * r + 16, :] = sw.transpose(0, 2, 1)

    # realign indices: rid[c][r][j] = pos_cr or 0, j in [0, 12800)
    NDP = ((P + 511) // 512) * 512  # padded dst cols (512-mult)
    rid_data = np.zeros((NC, 128, NDP // 16), dtype=np.int16)
    for c in range(NC):
        for r in range(NC):
            v = np.zeros(NDP, dtype=np.int16)
            v[:P] = pos_cr[c, r].astype(np.int16)
            rid_data[c, 16 * r:16 * r + 16, :] = v.reshape(NDP // 16, 16).T
    return dict(P=P, SL=SL, NCH=NCH, PW=PW, NDP=NDP, sched=sched,
                used=used_cols, idx_data=idx_data, rid_data=rid_data)


# ---------------------------------------------------------------------------
# Device program
# ---------------------------------------------------------------------------

def _ap(t_ap, offset, dims):
    return bass.AP(t_ap.tensor, t_ap.offset + offset, [list(t_ap.ap[0])] + dims)


def _build(N, F, HID, C, plan):
    P = plan["P"]
    NCH = plan["NCH"]
    PW = plan["PW"]
    NDP = plan["NDP"]
    sched = plan["sched"]
    used = plan["used"]
    NT2 = NDP // 128          # logits tiles
    KC = F // 128

    nc = bacc.Bacc(None, target_bir_lowering=False, debug=False,
                   num_devices=NC)

    xT_d = nc.dram_tensor("xT", [F, NDP], F32, kind="ExternalInput")
    w1_d = nc.dram_tensor("W1", [F, HID], F32, kind="ExternalInput")
    b1_d = nc.dram_tensor("b1c", [16, 1], F32, kind="ExternalInput")
    w2_d = nc.dram_tensor("W2r", [HID, C], F32, kind="ExternalInput")
    b2_d = nc.dram_tensor("b2r", [128, C], F32, kind="ExternalInput")
    m16_d = nc.dram_tensor("M16", [128, HID], F32, kind="ExternalInput")
    dinv_d = nc.dram_tensor("dinv16", [16, NDP], F32, kind="ExternalInput")
    idx_d = nc.dram_tensor("idxs", [128, NCH * (NI // 16)], I16,
                           kind="ExternalInput")
    rid_d = nc.dram_tensor("rids", [128, NDP // 16], I16,
                           kind="ExternalInput")
    out_d = nc.dram_tensor("out", [128, NT2 * C], F32, kind="ExternalOutput")

    with tile.TileContext(nc) as tc:
        with (
            tc.tile_pool(name="const", bufs=1) as cp,
            tc.tile_pool(name="dram", bufs=1, space="DRAM") as dp,
            tc.tile_pool(name="xt", bufs=3) as xtp,
            tc.tile_pool(name="zp", bufs=2, space="PSUM") as zpp,
            tc.tile_pool(name="zs", bufs=2) as zsp,
            tc.tile_pool(name="tab", bufs=1) as tbp,
            tc.tile_pool(name="idx", bufs=1) as ixp,
            tc.tile_pool(name="g", bufs=2) as gp,
            tc.tile_pool(name="part", bufs=1) as pp,
            tc.tile_pool(name="ra", bufs=3) as rap,
            tc.tile_pool(name="post", bufs=3) as pop,
            tc.tile_pool(name="lp", bufs=2, space="PSUM") as lpp,
        ):
            nc.gpsimd.load_library(library_config.ap_gather)

            w1 = []
            for kc in range(KC):
                t = cp.tile([128, HID], F32, tag=f"w1_{kc}")
                nc.sync.dma_start(out=t[:],
                                  in_=w1_d[kc * 128:(kc + 1) * 128, :])
                w1.append(t)
            b1c = cp.tile([16, 1], F32, tag="b1c")
            nc.sync.dma_start(out=b1c[:], in_=b1_d[:, :])
            w2r = cp.tile([HID, C], F32, tag="w2r")
            nc.sync.dma_start(out=w2r[:], in_=w2_d[:, :])
            b2r = cp.tile([128, C], F32, tag="b2r")
            nc.sync.dma_start(out=b2r[:], in_=b2_d[:, :])
            m16 = cp.tile([128, HID], F32, tag="m16")
            nc.sync.dma_start(out=m16[:], in_=m16_d[:, :])
            idxs = cp.tile([128, NCH * (NI // 16)], I16, tag="idxs")
            nc.sync.dma_start(out=idxs[:], in_=idx_d[:, :])
            rids = cp.tile([128, NDP // 16], I16, tag="rids")
            nc.sync.dma_start(out=rids[:], in_=rid_d[:, :])

            zT_dram = dp.tile([16, NDP], F32, tag="zT")
            h1_dram = dp.tile([16, NDP], F32, tag="h1T")
            zAG_dram = dp.tile([16, P], F32, tag="zAG")
            h1AG_dram = dp.tile([16, P], F32, tag="h1AG")
            tb1_dram = nc.dram_tensor("tb1s", [128, P], F32,
                                      kind="Internal", addr_space="Shared")
            tb2_dram = nc.dram_tensor("tb2s", [128, P], F32,
                                      kind="Internal", addr_space="Shared")

            # ---- z'^T = dinv * (x @ W1)^T, in 512-col chunks ----
            dma_engines = [nc.sync, nc.scalar]
            for j in range(NDP // 512):
                zp = zpp.tile([16, 512], F32, tag="zp")
                for kc in range(KC):
                    xa = xtp.tile([128, 512], F32, tag="xa")
                    dma_engines[(j * KC + kc) % 2].dma_start(
                        out=xa[:],
                        in_=xT_d[kc * 128:(kc + 1) * 128,
                                 j * 512:(j + 1) * 512])
                    nc.tensor.matmul(out=zp[:], lhsT=w1[kc][:], rhs=xa[:],
                                     start=(kc == 0), stop=(kc == KC - 1))
                dv = xtp.tile([16, 512], F32, tag="dv")
                nc.sync.dma_start(out=dv[:],
                                  in_=dinv_d[:, j * 512:(j + 1) * 512])
                zs = zsp.tile([16, 512], F32, tag="zs")
                nc.vector.tensor_tensor(out=zs[:], in0=zp[:], in1=dv[:],
                                        op=ALU.mult)
                nc.sync.dma_start(out=zT_dram[:, j * 512:(j + 1) * 512],
                                  in_=zs[:])

            nc.sync.dma_start(out=zAG_dram[:, :], in_=zT_dram[:, 0:P])
            nc.gpsimd.collective_compute(
                "AllGather", ALU.bypass,
                replica_groups=[list(range(NC))],
                ins=[zAG_dram[:, :]], outs=[tb1_dram[:, :]],
            )

            table = tbp.tile([128, P + 16], F32, tag="table")
            nc.vector.memset(table[:, P:P + 16], 0.0)
            partial = pp.tile([128, PW], F32, tag="partial")

            def layer(table_dram, self_dram, is_last):
                # spread the 6.4MB table load across engine DGE queues
                q = P // 4
                nc.sync.dma_start(out=table[:, 0:q],
                                  in_=table_dram[:, 0:q])
                nc.scalar.dma_start(out=table[:, q:2 * q],
                                    in_=table_dram[:, q:2 * q])
                nc.gpsimd.dma_start(out=table[:, 2 * q:3 * q],
                                    in_=table_dram[:, 2 * q:3 * q])
                nc.sync.dma_start(out=table[:, 3 * q:P],
                                  in_=table_dram[:, 3 * q:P])
                nc.vector.memset(partial[:], 0.0)
                for ch in range(NCH):
                    u = used[ch]
                    gt = gp.tile([128, NI], F32, tag="gt")
                    nc.gpsimd.ap_gather(
                        out_ap=gt[:, 0:u], in_ap=table[:],
                        idxs_ap=idxs[:, ch * (NI // 16):
                                     ch * (NI // 16) + u // 16],
                        channels=128, num_elems=P + 16, d=1, num_idxs=u,
                    )
                    for (col, k, nd, ppos) in sched[ch]:
                        nc.vector.tensor_reduce(
                            out=partial[:, ppos:ppos + nd],
                            in_=_ap(gt[:], col, [[k, nd], [1, k]]),
                            axis=mybir.AxisListType.X, op=ALU.add,
                        )
                # realign + combine + post, per 512-dst chunk
                RNI = 2048
                nrch = (NDP + RNI - 1) // RNI
                for rc in range(nrch):
                    w = min(RNI, NDP - rc * RNI)
                    ra = rap.tile([128, RNI], F32, tag="ra")
                    nc.gpsimd.ap_gather(
                        out_ap=ra[:, 0:w], in_ap=partial[:],
                        idxs_ap=rids[:, rc * (RNI // 16):
                                     rc * (RNI // 16) + w // 16],
                        channels=128, num_elems=PW, d=1, num_idxs=w,
                    )
                    for j in range(w // 512):
                        cols = slice(rc * RNI + j * 512,
                                     rc * RNI + j * 512 + 512)
                        ap_ = lpp.tile([16, 512], F32, tag="ap_")
                        nc.tensor.matmul(
                            out=ap_[:], lhsT=m16[:],
                            rhs=ra[:, j * 512:(j + 1) * 512],
                            start=True, stop=True)
                        sf = pop.tile([16, 512], F32, tag="sf")
                        nc.sync.dma_start(out=sf[:], in_=self_dram[:, cols])
                        dv = pop.tile([16, 512], F32, tag="dv2")
                        nc.sync.dma_start(out=dv[:], in_=dinv_d[:, cols])
                        ag = pop.tile([16, 512], F32, tag="ag")
                        nc.vector.tensor_tensor(out=ag[:], in0=ap_[:],
                                                in1=sf[:], op=ALU.add)
                        nc.vector.tensor_tensor(out=ag[:], in0=ag[:],
                                                in1=dv[:], op=ALU.mult)
                        if not is_last:
                            nc.vector.tensor_tensor(
                                out=ag[:], in0=ag[:],
                                in1=_ap(b1c[:], 0, [[0, 512]]), op=ALU.add)
                            nc.scalar.activation(out=ag[:], in_=ag[:],
                                                 func=AF.Relu)
                            nc.vector.tensor_tensor(out=ag[:], in0=ag[:],
                                                    in1=dv[:], op=ALU.mult)
                            nc.sync.dma_start(out=h1_dram[:, cols], in_=ag[:])
                        else:
                            # logits + log_softmax, 4x128 dsts batched
                            t2 = (rc * RNI + j * 512) // 128
                            lp = lpp.tile([128, 4 * C], F32, tag="lp")
                            for i in range(4):
                                nc.tensor.matmul(
                                    out=lp[:, i * C:(i + 1) * C],
                                    lhsT=ag[:, i * 128:(i + 1) * 128],
                                    rhs=w2r[:], start=True, stop=True)
                            lt = pop.tile([128, 4 * C], F32, tag="lt")
                            nc.vector.tensor_tensor(
                                out=lt[:], in0=lp[:],
                                in1=_ap(b2r[:], 0, [[0, 4], [1, C]]),
                                op=ALU.add)
                            nm = pop.tile([128, 4], F32, tag="nm")
                            nc.vector.tensor_reduce(
                                out=nm[:],
                                in_=_ap(lt[:], 0, [[C, 4], [1, C]]),
                                axis=mybir.AxisListType.X,
                                op=ALU.max, negate=True)
                            nc.vector.tensor_tensor(
                                out=lt[:], in0=lt[:],
                                in1=_ap(nm[:], 0, [[1, 4], [0, C]]),
                                op=ALU.add)
                            et = pop.tile([128, 4 * C], F32, tag="et")
                            nc.scalar.activation(out=et[:], in_=lt[:],
                                                 func=AF.Exp)
                            nc.vector.tensor_reduce(
                                out=nm[:],
                                in_=_ap(et[:], 0, [[C, 4], [1, C]]),
                                axis=mybir.AxisListType.X, op=ALU.add)
                            nc.scalar.activation(out=nm[:], in_=nm[:],
                                                 func=AF.Ln)
                            nc.vector.tensor_tensor(
                                out=lt[:], in0=lt[:],
                                in1=_ap(nm[:], 0, [[1, 4], [0, C]]),
                                op=ALU.subtract)
                            nc.sync.dma_start(
                                out=out_d[:, t2 * C:(t2 + 4) * C],
                                in_=lt[:])

            layer(tb1_dram, zT_dram, is_last=False)
            nc.sync.dma_start(out=h1AG_dram[:, :], in_=h1_dram[:, 0:P])
            nc.gpsimd.collective_compute(
                "AllGather", ALU.bypass,
                replica_groups=[list(range(NC))],
                ins=[h1AG_dram[:, :]], outs=[tb2_dram[:, :]],
            )
            layer(tb2_dram, h1_dram, is_last=True)

    return nc


# ---------------------------------------------------------------------------
# Entry point
# ---------------------------------------------------------------------------

def kernel(x, edge_index, W1, b1, W2, b2):
    N, F = x.shape
    HID = W1.shape[1]
    C = W2.shape[1]
    P = N // NC
    src = np.asarray(edge_index[0], dtype=np.int64)
    dst = np.asarray(edge_index[1], dtype=np.int64)

    deg = np.bincount(dst, minlength=N).astype(np.int64) + 1
    dinv = (1.0 / np.sqrt(deg.astype(np.float64))).astype(np.float32)

    plan = _make_plan(src, dst, N)
    NDP = plan["NDP"]
    nc = _build(N, F, HID, C, plan)

    x = np.asarray(x, dtype=np.float32)
    W1 = np.ascontiguousarray(np.asarray(W1, dtype=np.float32))
    W2 = np.ascontiguousarray(np.asarray(W2, dtype=np.float32))
    b2r = np.tile(np.asarray(b2, dtype=np.float32)[None, :], (128, 1))
    M16 = np.zeros((128, HID), dtype=np.float32)
    for r in range(NC):
        M16[16 * r:16 * r + 16, :] = np.eye(HID, dtype=np.float32)

    in_maps = []
    for c in range(NC):
        xT = np.zeros((F, NDP), dtype=np.float32)
        xT[:, :P] = x[c * P:(c + 1) * P].T
        d16 = np.zeros((16, NDP), dtype=np.float32)
        d16[:, :P] = dinv[c * P:(c + 1) * P][None, :]
        in_maps.append({
            "xT": np.ascontiguousarray(xT),
            "W1": W1,
            "b1c": np.ascontiguousarray(
                np.asarray(b1, np.float32).reshape(16, 1)),
            "W2r": W2,
            "b2r": b2r,
            "M16": M16,
            "dinv16": np.ascontiguousarray(d16),
            "idxs": np.ascontiguousarray(
                plan["idx_data"][c].transpose(1, 0, 2).reshape(128, -1)),
            "rids": np.ascontiguousarray(plan["rid_data"][c]),
        })

    trace = bool(int(os.environ.get("GCN_TRACE", "0")))
    if int(os.environ.get("GCN_SIM", "0")):
        from concourse.bass_interp import MultiCoreSim

        sim = MultiCoreSim(nc, num_cores=NC, trace=False)
        for c, core in enumerate(sim.cores.values()):
            for k, v in in_maps[c].items():
                core.tensor(k)[:] = v
        sim.simulate(check_with_hw=False)
        results = [{"out": np.array(core.tensor("out"))}
                   for core in sim.cores.values()]
        _last_result["exec_time_ns"] = None
    else:
        nc.finalize()
        br = bass_utils.run_bass_kernel_spmd(
            nc, in_maps, core_ids=list(range(NC)), trace=trace,
        )
        results = br.results
        _last_result["exec_time_ns"] = br.exec_time_ns

    _last_result["results"] = results
    _last_result["plan"] = plan

    out = np.empty((N, C), dtype=np.float32)
    for c in range(NC):
        arr = results[c]["out"].reshape(128, NDP // 128, C)
        arr = arr.transpose(1, 0, 2).reshape(NDP, C)
        out[c * P:(c + 1) * P] = arr[:P]
    return out



# revision 13
# speedup vs baseline: 1.0859x; 1.0101x over previous
"""GCN v3: ap_gather-based edge gather on 8 TRN2 cores.

Layout: nodes sharded 8 ways (core c owns dst range [cP,(c+1)P)).  Tables
live transposed in SBUF as [128 = 8 src-ranges x 16 feats, P nodes]; each
16-partition GPSIMD group gathers edges whose src falls in its range
(ap_gather, group-private int16 index lists).  Per (dst, range) segment
sums run on DVE (K-run reduces, K-desc order, SPMD-global structure);
partials are realigned to global dst order by a second ap_gather and
summed across ranges by one PE matmul.  Self-loop terms are added
directly from the core's own z'/h1' columns (no gather).  Both layers
share one index/schedule set since the graph is identical.

v3 over v2 (4.07ms -> 3.79ms):
 - suffix-max slot schedule with zero-pad column: a class-k dst may sit in
   a K'>=k slot padded with gathers of a zeroed table column, so slot
   capacities cover suffix maxima over (core,range) pairs instead of
   per-class maxima (~53.5k -> ~50.8k gather columns per group per layer;
   ap_gather costs ~27ns per index on each of the 8 DSPs, which is the
   kernel's dominant cost).
 - log_softmax tail batched 4x128 dsts wide (one DVE/Act chain per 512).
 - AllGather outputs in Shared DRAM scratchpad (fast collective path).
 - table/x loads spread across sync/scalar/gpsimd DGE queues.
"""

import os
import sys

for _p in ("/opt/trn_rl_repo", "/opt/pypackages"):
    if _p not in sys.path:
        sys.path.insert(0, _p)

import ml_dtypes
import numpy as np

from concourse import bacc, bass, tile, mybir, library_config
from concourse import bass_utils

F32 = mybir.dt.float32
BF16 = mybir.dt.bfloat16
I16 = mybir.dt.int16
AF = mybir.ActivationFunctionType
ALU = mybir.AluOpType

NC = 8
NI = 4096          # gather columns per ap_gather call

_last_result = {}


# ---------------------------------------------------------------------------
# Host-side plan
# ---------------------------------------------------------------------------

def _make_plan(src, dst, N):
    P = N // NC
    ZC = P  # zero column index (table has 16 zeroed pad columns at P..P+15)
    core_d = dst // P
    rng_s = src // P
    dloc = dst - core_d * P
    sloc = src - rng_s * P

    # per (core, range): dst counts
    K_cr = []        # K_cr[c][r] = dict-like arrays: (dsts_sorted, counts)
    KMAX = 0
    for c in range(NC):
        row = []
        mc = core_d == c
        for r in range(NC):
            m = mc & (rng_s == r)
            d_ = dloc[m]
            s_ = sloc[m]
            cnt = np.bincount(d_, minlength=P)
            KMAX = max(KMAX, int(cnt.max()))
            row.append((d_, s_, cnt))
        K_cr.append(row)

    # ONE slot-structure shared by all (core, range) pairs so every reduce is
    # full-128-partition.  A class-k dst may occupy a K'>=k slot, padding the
    # run with gathers of the zero column, so capacities only need to cover
    # the suffix maxima (near-zero padding) instead of per-class maxima.
    sufmax = np.zeros(KMAX + 2, dtype=np.int64)
    for c in range(NC):
        for r in range(NC):
            cnt = K_cr[c][r][2]
            ks, nds = np.unique(cnt[cnt > 0], return_counts=True)
            cc = np.zeros(KMAX + 2, dtype=np.int64)
            cc[ks] = nds
            suf = cc[::-1].cumsum()[::-1]
            np.maximum(sufmax, suf, out=sufmax)
    nd_g = sufmax - np.append(sufmax[1:], 0)
    struct = [(k, int(nd_g[k])) for k in range(KMAX, 0, -1) if nd_g[k] > 0]

    # chunked schedule: entries (coloff, K, nd, ppos); runs never straddle
    # chunk boundaries; identical for every class/core.
    sched = []
    ch = 0
    col = 0
    ppos = 1
    for (k, nd) in struct:
        left = nd
        while left > 0:
            while ch >= len(sched):
                sched.append([])
            fit = min(left, (NI - col) // k)
            if fit == 0:
                ch += 1
                col = 0
                continue
            sched[ch].append((col, k, fit, ppos))
            col += fit * k
            ppos += fit
            left -= fit
    NCH = len(sched)
    SL = NCH * NI
    PW = ppos + 2 - (ppos % 2)  # even pad
    # per-chunk used columns (mult of 16): trim the gather of the tail chunk
    used_cols = []
    for ch_e in sched:
        u = max(col + k * nd for (col, k, nd, _) in ch_e)
        used_cols.append(min(NI, ((u + 15) // 16) * 16))

    # per-core idx streams + partial position of each (dst, r)
    idx_data = np.full((NC, NCH, 128, NI // 16), ZC, dtype=np.int16)
    pos_cr = np.full((NC, NC, P), 0, dtype=np.int32)  # [c][r][dst] -> ppos
    for c in range(NC):
        for r in range(NC):
            d_, s_, cnt = K_cr[c][r]
            order = np.lexsort((s_, d_))
            d_s = d_[order]
            s_s = s_[order]
            starts = np.searchsorted(d_s, np.arange(P))
            ends = np.searchsorted(d_s, np.arange(P), side="right")
            # all dsts with count>0, sorted by count desc (dst asc within)
            nz = np.where(cnt > 0)[0]
            queue = nz[np.argsort(-cnt[nz], kind="stable")]
            qi = 0
            stream = np.full(NCH * NI, ZC, dtype=np.int16)
            # walk the same schedule the device uses (slots in K-desc order)
            for ch in range(NCH):
                base = ch * NI
                for (col, k, fit, ppos) in sched[ch]:
                    take = queue[qi:qi + fit]
                    qi += len(take)
                    for j, dd in enumerate(take):
                        kk = ends[dd] - starts[dd]
                        assert kk <= k, (kk, k)
                        sl = s_s[starts[dd]:ends[dd]]
                        o = base + col + j * k
                        stream[o:o + kk] = sl
                        pos_cr[c, r, dd] = ppos + j
            assert qi == len(queue), (qi, len(queue))
            # wrap into tiles: position i -> [16r + i%16, i//16]
            sw = stream.reshape(NCH, NI // 16, 16)
            idx_data[c, :, 16 * r:16 * r + 16, :] = sw.transpose(0, 2, 1)

    # realign indices: rid[c][r][j] = pos_cr or 0, j in [0, 12800)
    NDP = ((P + 511) // 512) * 512  # padded dst cols (512-mult)
    rid_data = np.zeros((NC, 128, NDP // 16), dtype=np.int16)
    for c in range(NC):
        for r in range(NC):
            v = np.zeros(NDP, dtype=np.int16)
            v[:P] = pos_cr[c, r].astype(np.int16)
            rid_data[c, 16 * r:16 * r + 16, :] = v.reshape(NDP // 16, 16).T
    return dict(P=P, SL=SL, NCH=NCH, PW=PW, NDP=NDP, sched=sched,
                used=used_cols, idx_data=idx_data, rid_data=rid_data)


# ---------------------------------------------------------------------------
# Device program
# ---------------------------------------------------------------------------

def _ap(t_ap, offset, dims):
    return bass.AP(t_ap.tensor, t_ap.offset + offset, [list(t_ap.ap[0])] + dims)


def _build(N, F, HID, C, plan):
    P = plan["P"]
    NCH = plan["NCH"]
    PW = plan["PW"]
    NDP = plan["NDP"]
    sched = plan["sched"]
    used = plan["used"]
    NT2 = NDP // 128          # logits tiles
    KC = F // 128

    nc = bacc.Bacc(None, target_bir_lowering=False, debug=False,
                   num_devices=NC)

    xT_d = nc.dram_tensor("xT", [F, NDP], BF16, kind="ExternalInput")
    w1_d = nc.dram_tensor("W1", [F, HID], BF16, kind="ExternalInput")
    b1_d = nc.dram_tensor("b1c", [16, 1], F32, kind="ExternalInput")
    w2_d = nc.dram_tensor("W2r", [HID, C], F32, kind="ExternalInput")
    b2_d = nc.dram_tensor("b2r", [128, C], F32, kind="ExternalInput")
    m16_d = nc.dram_tensor("M16", [128, HID], F32, kind="ExternalInput")
    dinv_d = nc.dram_tensor("dinv16", [16, NDP], F32, kind="ExternalInput")
    idx_d = nc.dram_tensor("idxs", [128, NCH * (NI // 16)], I16,
                           kind="ExternalInput")
    rid_d = nc.dram_tensor("rids", [128, NDP // 16], I16,
                           kind="ExternalInput")
    out_d = nc.dram_tensor("out", [128, NT2 * C], F32, kind="ExternalOutput")

    with tile.TileContext(nc) as tc:
        with (
            tc.tile_pool(name="const", bufs=1) as cp,
            tc.tile_pool(name="dram", bufs=1, space="DRAM") as dp,
            tc.tile_pool(name="xt", bufs=3) as xtp,
            tc.tile_pool(name="zp", bufs=2, space="PSUM") as zpp,
            tc.tile_pool(name="zs", bufs=2) as zsp,
            tc.tile_pool(name="tab", bufs=1) as tbp,
            tc.tile_pool(name="idx", bufs=1) as ixp,
            tc.tile_pool(name="g", bufs=2) as gp,
            tc.tile_pool(name="part", bufs=1) as pp,
            tc.tile_pool(name="ra", bufs=3) as rap,
            tc.tile_pool(name="post", bufs=3) as pop,
            tc.tile_pool(name="lp", bufs=2, space="PSUM") as lpp,
        ):
            nc.gpsimd.load_library(library_config.ap_gather)

            w1 = []
            for kc in range(KC):
                t = cp.tile([128, HID], BF16, tag=f"w1_{kc}")
                nc.sync.dma_start(out=t[:],
                                  in_=w1_d[kc * 128:(kc + 1) * 128, :])
                w1.append(t)
            b1c = cp.tile([16, 1], F32, tag="b1c")
            nc.sync.dma_start(out=b1c[:], in_=b1_d[:, :])
            w2r = cp.tile([HID, C], F32, tag="w2r")
            nc.sync.dma_start(out=w2r[:], in_=w2_d[:, :])
            b2r = cp.tile([128, C], F32, tag="b2r")
            nc.sync.dma_start(out=b2r[:], in_=b2_d[:, :])
            m16 = cp.tile([128, HID], F32, tag="m16")
            nc.sync.dma_start(out=m16[:], in_=m16_d[:, :])
            idxs = cp.tile([128, NCH * (NI // 16)], I16, tag="idxs")
            nc.sync.dma_start(out=idxs[:], in_=idx_d[:, :])
            rids = cp.tile([128, NDP // 16], I16, tag="rids")
            nc.sync.dma_start(out=rids[:], in_=rid_d[:, :])

            zT_dram = dp.tile([16, NDP], BF16, tag="zT")
            h1_dram = dp.tile([16, NDP], BF16, tag="h1T")
            zAG_dram = dp.tile([16, P], BF16, tag="zAG")
            h1AG_dram = dp.tile([16, P], BF16, tag="h1AG")
            tb1_dram = nc.dram_tensor("tb1s", [128, P], BF16,
                                      kind="Internal", addr_space="Shared")
            tb2_dram = nc.dram_tensor("tb2s", [128, P], BF16,
                                      kind="Internal", addr_space="Shared")

            # ---- z'^T = dinv * (x @ W1)^T, in 512-col chunks ----
            dma_engines = [nc.sync, nc.scalar]
            for j in range(NDP // 512):
                zp = zpp.tile([16, 512], F32, tag="zp")
                for kc in range(KC):
                    xa = xtp.tile([128, 512], BF16, tag="xa")
                    dma_engines[(j * KC + kc) % 2].dma_start(
                        out=xa[:],
                        in_=xT_d[kc * 128:(kc + 1) * 128,
                                 j * 512:(j + 1) * 512])
                    nc.tensor.matmul(out=zp[:], lhsT=w1[kc][:], rhs=xa[:],
                                     start=(kc == 0), stop=(kc == KC - 1))
                dv = xtp.tile([16, 512], F32, tag="dv")
                nc.sync.dma_start(out=dv[:],
                                  in_=dinv_d[:, j * 512:(j + 1) * 512])
                zs = zsp.tile([16, 512], BF16, tag="zs")
                nc.vector.tensor_tensor(out=zs[:], in0=zp[:], in1=dv[:],
                                        op=ALU.mult)
                nc.sync.dma_start(out=zT_dram[:, j * 512:(j + 1) * 512],
                                  in_=zs[:])

            nc.sync.dma_start(out=zAG_dram[:, :], in_=zT_dram[:, 0:P])
            nc.gpsimd.collective_compute(
                "AllGather", ALU.bypass,
                replica_groups=[list(range(NC))],
                ins=[zAG_dram[:, :]], outs=[tb1_dram[:, :]],
            )

            table = tbp.tile([128, P + 16], F32, tag="table")
            tableB = tbp.tile([128, P], BF16, tag="tableB")
            nc.vector.memset(table[:, P:P + 16], 0.0)
            partial = pp.tile([128, PW], F32, tag="partial")

            def layer(table_dram, self_dram, is_last):
                # spread the 3.2MB bf16 table load across engine DGE
                # queues, then widen to the fp32 gather table on DVE
                q = P // 4
                nc.sync.dma_start(out=tableB[:, 0:q],
                                  in_=table_dram[:, 0:q])
                nc.scalar.dma_start(out=tableB[:, q:2 * q],
                                    in_=table_dram[:, q:2 * q])
                nc.gpsimd.dma_start(out=tableB[:, 2 * q:3 * q],
                                    in_=table_dram[:, 2 * q:3 * q])
                nc.sync.dma_start(out=tableB[:, 3 * q:P],
                                  in_=table_dram[:, 3 * q:P])
                nc.vector.tensor_scalar(out=table[:, 0:P], in0=tableB[:],
                                        scalar1=1.0, scalar2=None,
                                        op0=ALU.mult)
                nc.vector.memset(partial[:], 0.0)
                for ch in range(NCH):
                    u = used[ch]
                    gt = gp.tile([128, NI], F32, tag="gt")
                    nc.gpsimd.ap_gather(
                        out_ap=gt[:, 0:u], in_ap=table[:],
                        idxs_ap=idxs[:, ch * (NI // 16):
                                     ch * (NI // 16) + u // 16],
                        channels=128, num_elems=P + 16, d=1, num_idxs=u,
                    )
                    for (col, k, nd, ppos) in sched[ch]:
                        nc.vector.tensor_reduce(
                            out=partial[:, ppos:ppos + nd],
                            in_=_ap(gt[:], col, [[k, nd], [1, k]]),
                            axis=mybir.AxisListType.X, op=ALU.add,
                        )
                # realign + combine + post, per 512-dst chunk
                RNI = 1024
                nrch = (NDP + RNI - 1) // RNI
                for rc in range(nrch):
                    w = min(RNI, NDP - rc * RNI)
                    ra = rap.tile([128, RNI], F32, tag="ra")
                    nc.gpsimd.ap_gather(
                        out_ap=ra[:, 0:w], in_ap=partial[:],
                        idxs_ap=rids[:, rc * (RNI // 16):
                                     rc * (RNI // 16) + w // 16],
                        channels=128, num_elems=PW, d=1, num_idxs=w,
                    )
                    for j in range(w // 512):
                        cols = slice(rc * RNI + j * 512,
                                     rc * RNI + j * 512 + 512)
                        ap_ = lpp.tile([16, 512], F32, tag="ap_")
                        nc.tensor.matmul(
                            out=ap_[:], lhsT=m16[:],
                            rhs=ra[:, j * 512:(j + 1) * 512],
                            start=True, stop=True)
                        sf = pop.tile([16, 512], BF16, tag="sf")
                        nc.sync.dma_start(out=sf[:], in_=self_dram[:, cols])
                        dv = pop.tile([16, 512], F32, tag="dv2")
                        nc.sync.dma_start(out=dv[:], in_=dinv_d[:, cols])
                        ag = pop.tile([16, 512], F32, tag="ag")
                        nc.vector.tensor_tensor(out=ag[:], in0=ap_[:],
                                                in1=sf[:], op=ALU.add)
                        nc.vector.tensor_tensor(out=ag[:], in0=ag[:],
                                                in1=dv[:], op=ALU.mult)
                        if not is_last:
                            nc.vector.tensor_tensor(
                                out=ag[:], in0=ag[:],
                                in1=_ap(b1c[:], 0, [[0, 512]]), op=ALU.add)
                            nc.scalar.activation(out=ag[:], in_=ag[:],
                                                 func=AF.Relu)
                            hb = pop.tile([16, 512], BF16, tag="hb")
                            nc.vector.tensor_tensor(out=hb[:], in0=ag[:],
                                                    in1=dv[:], op=ALU.mult)
                            nc.sync.dma_start(out=h1_dram[:, cols], in_=hb[:])
                        else:
                            # logits + log_softmax, 4x128 dsts batched
                            t2 = (rc * RNI + j * 512) // 128
                            lp = lpp.tile([128, 4 * C], F32, tag="lp")
                            for i in range(4):
                                nc.tensor.matmul(
                                    out=lp[:, i * C:(i + 1) * C],
                                    lhsT=ag[:, i * 128:(i + 1) * 128],
                                    rhs=w2r[:], start=True, stop=True)
                            lt = pop.tile([128, 4 * C], F32, tag="lt")
                            nc.vector.tensor_tensor(
                                out=lt[:], in0=lp[:],
                                in1=_ap(b2r[:], 0, [[0, 4], [1, C]]),
                                op=ALU.add)
                            nm = pop.tile([128, 4], F32, tag="nm")
                            nc.vector.tensor_reduce(
                                out=nm[:],
                                in_=_ap(lt[:], 0, [[C, 4], [1, C]]),
                                axis=mybir.AxisListType.X,
                                op=ALU.max, negate=True)
                            nc.vector.tensor_tensor(
                                out=lt[:], in0=lt[:],
                                in1=_ap(nm[:], 0, [[1, 4], [0, C]]),
                                op=ALU.add)
                            et = pop.tile([128, 4 * C], F32, tag="et")
                            nc.scalar.activation(out=et[:], in_=lt[:],
                                                 func=AF.Exp)
                            nc.vector.tensor_reduce(
                                out=nm[:],
                                in_=_ap(et[:], 0, [[C, 4], [1, C]]),
                                axis=mybir.AxisListType.X, op=ALU.add)
                            nc.scalar.activation(out=nm[:], in_=nm[:],
                                                 func=AF.Ln)
                            nc.vector.tensor_tensor(
                                out=lt[:], in0=lt[:],
                                in1=_ap(nm[:], 0, [[1, 4], [0, C]]),
                                op=ALU.subtract)
                            nc.sync.dma_start(
                                out=out_d[:, t2 * C:(t2 + 4) * C],
                                in_=lt[:])

            layer(tb1_dram, zT_dram, is_last=False)
            nc.sync.dma_start(out=h1AG_dram[:, :], in_=h1_dram[:, 0:P])
            nc.gpsimd.collective_compute(
                "AllGather", ALU.bypass,
                replica_groups=[list(range(NC))],
                ins=[h1AG_dram[:, :]], outs=[tb2_dram[:, :]],
            )
            layer(tb2_dram, h1_dram, is_last=True)

    return nc


# ---------------------------------------------------------------------------
# Entry point
# ---------------------------------------------------------------------------

def kernel(x, edge_index, W1, b1, W2, b2):
    N, F = x.shape
    HID = W1.shape[1]
    C = W2.shape[1]
    P = N // NC
    src = np.asarray(edge_index[0], dtype=np.int64)
    dst = np.asarray(edge_index[1], dtype=np.int64)

    deg = np.bincount(dst, minlength=N).astype(np.int64) + 1
    dinv = (1.0 / np.sqrt(deg.astype(np.float64))).astype(np.float32)

    plan = _make_plan(src, dst, N)
    NDP = plan["NDP"]
    nc = _build(N, F, HID, C, plan)

    x = np.asarray(x, dtype=np.float32)
    W1 = np.ascontiguousarray(
        np.asarray(W1, dtype=np.float32).astype(ml_dtypes.bfloat16))
    W2 = np.ascontiguousarray(np.asarray(W2, dtype=np.float32))
    b2r = np.tile(np.asarray(b2, dtype=np.float32)[None, :], (128, 1))
    M16 = np.zeros((128, HID), dtype=np.float32)
    for r in range(NC):
        M16[16 * r:16 * r + 16, :] = np.eye(HID, dtype=np.float32)

    in_maps = []
    for c in range(NC):
        xT = np.zeros((F, NDP), dtype=ml_dtypes.bfloat16)
        xT[:, :P] = x[c * P:(c + 1) * P].T.astype(ml_dtypes.bfloat16)
        d16 = np.zeros((16, NDP), dtype=np.float32)
        d16[:, :P] = dinv[c * P:(c + 1) * P][None, :]
        in_maps.append({
            "xT": np.ascontiguousarray(xT),
            "W1": W1,
            "b1c": np.ascontiguousarray(
                np.asarray(b1, np.float32).reshape(16, 1)),
            "W2r": W2,
            "b2r": b2r,
            "M16": M16,
            "dinv16": np.ascontiguousarray(d16),
            "idxs": np.ascontiguousarray(
                plan["idx_data"][c].transpose(1, 0, 2).reshape(128, -1)),
            "rids": np.ascontiguousarray(plan["rid_data"][c]),
        })

    trace = bool(int(os.environ.get("GCN_TRACE", "0")))
    if int(os.environ.get("GCN_SIM", "0")):
        from concourse.bass_interp import MultiCoreSim

        sim = MultiCoreSim(nc, num_cores=NC, trace=False)
        for c, core in enumerate(sim.cores.values()):
            for k, v in in_maps[c].items():
                core.tensor(k)[:] = v
        sim.simulate(check_with_hw=False)
        results = [{"out": np.array(core.tensor("out"))}
                   for core in sim.cores.values()]
        _last_result["exec_time_ns"] = None
    else:
        nc.finalize()
        br = bass_utils.run_bass_kernel_spmd(
            nc, in_maps, core_ids=list(range(NC)), trace=trace,
        )
        results = br.results
        _last_result["exec_time_ns"] = br.exec_time_ns

    _last_result["results"] = results
    _last_result["plan"] = plan

    out = np.empty((N, C), dtype=np.float32)
    for c in range(NC):
        arr = results[c]["out"].reshape(128, NDP // 128, C)
        arr = arr.transpose(1, 0, 2).reshape(NDP, C)
        out[c * P:(c + 1) * P] = arr[:P]
    return out



# revision 14
# speedup vs baseline: 1.0958x; 1.0091x over previous
"""GCN v3: ap_gather-based edge gather on 8 TRN2 cores.

Layout: nodes sharded 8 ways (core c owns dst range [cP,(c+1)P)).  Tables
live transposed in SBUF as [128 = 8 src-ranges x 16 feats, P nodes]; each
16-partition GPSIMD group gathers edges whose src falls in its range
(ap_gather, group-private int16 index lists).  Per (dst, range) segment
sums run on DVE (K-run reduces, K-desc order, SPMD-global structure);
partials are realigned to global dst order by a second ap_gather and
summed across ranges by one PE matmul.  Self-loop terms are added
directly from the core's own z'/h1' columns (no gather).  Both layers
share one index/schedule set since the graph is identical.

v3 over v2 (4.07ms -> 3.79ms):
 - suffix-max slot schedule with zero-pad column: a class-k dst may sit in
   a K'>=k slot padded with gathers of a zeroed table column, so slot
   capacities cover suffix maxima over (core,range) pairs instead of
   per-class maxima (~53.5k -> ~50.8k gather columns per group per layer;
   ap_gather costs ~27ns per index on each of the 8 DSPs, which is the
   kernel's dominant cost).
 - log_softmax tail batched 4x128 dsts wide (one DVE/Act chain per 512).
 - AllGather outputs in Shared DRAM scratchpad (fast collective path).
 - table/x loads spread across sync/scalar/gpsimd DGE queues.
"""

import os
import sys

for _p in ("/opt/trn_rl_repo", "/opt/pypackages"):
    if _p not in sys.path:
        sys.path.insert(0, _p)

import ml_dtypes
import numpy as np

from concourse import bacc, bass, tile, mybir, library_config
from concourse import bass_utils

F32 = mybir.dt.float32
BF16 = mybir.dt.bfloat16
I16 = mybir.dt.int16
AF = mybir.ActivationFunctionType
ALU = mybir.AluOpType

NC = 8
NI = 4096          # gather columns per ap_gather call

_last_result = {}


# ---------------------------------------------------------------------------
# Host-side plan
# ---------------------------------------------------------------------------

def _make_plan(src, dst, N):
    P = N // NC
    ZC = P  # zero column index (table has 16 zeroed pad columns at P..P+15)
    core_d = dst // P
    rng_s = src // P
    dloc = dst - core_d * P
    sloc = src - rng_s * P

    # per (core, range): dst counts
    K_cr = []        # K_cr[c][r] = dict-like arrays: (dsts_sorted, counts)
    KMAX = 0
    for c in range(NC):
        row = []
        mc = core_d == c
        for r in range(NC):
            m = mc & (rng_s == r)
            d_ = dloc[m]
            s_ = sloc[m]
            cnt = np.bincount(d_, minlength=P)
            KMAX = max(KMAX, int(cnt.max()))
            row.append((d_, s_, cnt))
        K_cr.append(row)

    # ONE slot-structure shared by all (core, range) pairs so every reduce is
    # full-128-partition.  A class-k dst may occupy a K'>=k slot, padding the
    # run with gathers of the zero column, so capacities only need to cover
    # the suffix maxima (near-zero padding) instead of per-class maxima.
    sufmax = np.zeros(KMAX + 2, dtype=np.int64)
    for c in range(NC):
        for r in range(NC):
            cnt = K_cr[c][r][2]
            ks, nds = np.unique(cnt[cnt > 0], return_counts=True)
            cc = np.zeros(KMAX + 2, dtype=np.int64)
            cc[ks] = nds
            suf = cc[::-1].cumsum()[::-1]
            np.maximum(sufmax, suf, out=sufmax)
    nd_g = sufmax - np.append(sufmax[1:], 0)
    struct = [(k, int(nd_g[k])) for k in range(KMAX, 0, -1) if nd_g[k] > 0]

    # chunked schedule: entries (coloff, K, nd, ppos); runs never straddle
    # chunk boundaries; identical for every class/core.
    sched = []
    ch = 0
    col = 0
    ppos = 1
    for (k, nd) in struct:
        left = nd
        while left > 0:
            while ch >= len(sched):
                sched.append([])
            fit = min(left, (NI - col) // k)
            if fit == 0:
                ch += 1
                col = 0
                continue
            sched[ch].append((col, k, fit, ppos))
            col += fit * k
            ppos += fit
            left -= fit
    NCH = len(sched)
    SL = NCH * NI
    PW = ppos + 2 - (ppos % 2)  # even pad
    # per-chunk used columns (mult of 16): trim the gather of the tail chunk
    used_cols = []
    for ch_e in sched:
        u = max(col + k * nd for (col, k, nd, _) in ch_e)
        used_cols.append(min(NI, ((u + 15) // 16) * 16))

    # per-core idx streams + partial position of each (dst, r)
    idx_data = np.full((NC, NCH, 128, NI // 16), ZC, dtype=np.int16)
    pos_cr = np.full((NC, NC, P), 0, dtype=np.int32)  # [c][r][dst] -> ppos
    for c in range(NC):
        for r in range(NC):
            d_, s_, cnt = K_cr[c][r]
            order = np.lexsort((s_, d_))
            d_s = d_[order]
            s_s = s_[order]
            starts = np.searchsorted(d_s, np.arange(P))
            ends = np.searchsorted(d_s, np.arange(P), side="right")
            # all dsts with count>0, sorted by count desc (dst asc within)
            nz = np.where(cnt > 0)[0]
            queue = nz[np.argsort(-cnt[nz], kind="stable")]
            qi = 0
            stream = np.full(NCH * NI, ZC, dtype=np.int16)
            # walk the same schedule the device uses (slots in K-desc order)
            for ch in range(NCH):
                base = ch * NI
                for (col, k, fit, ppos) in sched[ch]:
                    take = queue[qi:qi + fit]
                    qi += len(take)
                    for j, dd in enumerate(take):
                        kk = ends[dd] - starts[dd]
                        assert kk <= k, (kk, k)
                        sl = s_s[starts[dd]:ends[dd]]
                        o = base + col + j * k
                        stream[o:o + kk] = sl
                        pos_cr[c, r, dd] = ppos + j
            assert qi == len(queue), (qi, len(queue))
            # wrap into tiles: position i -> [16r + i%16, i//16]
            sw = stream.reshape(NCH, NI // 16, 16)
            idx_data[c, :, 16 * r:16 * r + 16, :] = sw.transpose(0, 2, 1)

    # realign indices: rid[c][r][j] = pos_cr or 0, j in [0, 12800)
    NDP = ((P + 511) // 512) * 512  # padded dst cols (512-mult)
    rid_data = np.zeros((NC, 128, NDP // 16), dtype=np.int16)
    for c in range(NC):
        for r in range(NC):
            v = np.zeros(NDP, dtype=np.int16)
            v[:P] = pos_cr[c, r].astype(np.int16)
            rid_data[c, 16 * r:16 * r + 16, :] = v.reshape(NDP // 16, 16).T
    return dict(P=P, SL=SL, NCH=NCH, PW=PW, NDP=NDP, sched=sched,
                used=used_cols, idx_data=idx_data, rid_data=rid_data)


# ---------------------------------------------------------------------------
# Device program
# ---------------------------------------------------------------------------

def _ap(t_ap, offset, dims):
    return bass.AP(t_ap.tensor, t_ap.offset + offset, [list(t_ap.ap[0])] + dims)


def _build(N, F, HID, C, plan):
    P = plan["P"]
    NCH = plan["NCH"]
    PW = plan["PW"]
    NDP = plan["NDP"]
    sched = plan["sched"]
    used = plan["used"]
    NT2 = NDP // 128          # logits tiles
    KC = F // 128

    nc = bacc.Bacc(None, target_bir_lowering=False, debug=False,
                   num_devices=NC)

    xT_d = nc.dram_tensor("xT", [F, NDP], BF16, kind="ExternalInput")
    w1_d = nc.dram_tensor("W1", [F, HID], BF16, kind="ExternalInput")
    b1_d = nc.dram_tensor("b1c", [16, 1], F32, kind="ExternalInput")
    w2_d = nc.dram_tensor("W2r", [HID, C], F32, kind="ExternalInput")
    b2_d = nc.dram_tensor("b2r", [128, C], F32, kind="ExternalInput")
    m16_d = nc.dram_tensor("M16", [128, HID], F32, kind="ExternalInput")
    dinv_d = nc.dram_tensor("dinv16", [16, NDP], F32, kind="ExternalInput")
    idx_d = nc.dram_tensor("idxs", [128, NCH * (NI // 16)], I16,
                           kind="ExternalInput")
    rid_d = nc.dram_tensor("rids", [128, NDP // 16], I16,
                           kind="ExternalInput")
    out_d = nc.dram_tensor("out", [128, NT2 * C], F32, kind="ExternalOutput")

    with tile.TileContext(nc) as tc:
        with (
            tc.tile_pool(name="const", bufs=1) as cp,
            tc.tile_pool(name="dram", bufs=1, space="DRAM") as dp,
            tc.tile_pool(name="xt", bufs=3) as xtp,
            tc.tile_pool(name="zp", bufs=2, space="PSUM") as zpp,
            tc.tile_pool(name="zs", bufs=2) as zsp,
            tc.tile_pool(name="tab", bufs=1) as tbp,
            tc.tile_pool(name="idx", bufs=1) as ixp,
            tc.tile_pool(name="g", bufs=2) as gp,
            tc.tile_pool(name="part", bufs=1) as pp,
            tc.tile_pool(name="ra", bufs=3) as rap,
            tc.tile_pool(name="post", bufs=3) as pop,
            tc.tile_pool(name="lp", bufs=2, space="PSUM") as lpp,
        ):
            nc.gpsimd.load_library(library_config.ap_gather)

            w1 = []
            for kc in range(KC):
                t = cp.tile([128, HID], BF16, tag=f"w1_{kc}")
                nc.sync.dma_start(out=t[:],
                                  in_=w1_d[kc * 128:(kc + 1) * 128, :])
                w1.append(t)
            b1c = cp.tile([16, 1], F32, tag="b1c")
            nc.sync.dma_start(out=b1c[:], in_=b1_d[:, :])
            w2r = cp.tile([HID, C], F32, tag="w2r")
            nc.sync.dma_start(out=w2r[:], in_=w2_d[:, :])
            b2r = cp.tile([128, C], F32, tag="b2r")
            nc.sync.dma_start(out=b2r[:], in_=b2_d[:, :])
            m16 = cp.tile([128, HID], F32, tag="m16")
            nc.sync.dma_start(out=m16[:], in_=m16_d[:, :])
            idxs = cp.tile([128, NCH * (NI // 16)], I16, tag="idxs")
            nc.sync.dma_start(out=idxs[:], in_=idx_d[:, :])
            rids = cp.tile([128, NDP // 16], I16, tag="rids")
            nc.sync.dma_start(out=rids[:], in_=rid_d[:, :])

            zT_dram = dp.tile([16, NDP], BF16, tag="zT")
            h1_dram = dp.tile([16, NDP], BF16, tag="h1T")
            # half-split collectives: first half AllGathers while the tail
            # of the producing phase still runs
            HA = (P // 1024) * 512 if P >= 4096 else P
            HB = P - HA
            zAGa = dp.tile([16, HA], BF16, tag="zAGa")
            h1AGa = dp.tile([16, HA], BF16, tag="h1AGa")
            tb1a = nc.dram_tensor("tb1a", [128, HA], BF16,
                                  kind="Internal", addr_space="Shared")
            tb2a = nc.dram_tensor("tb2a", [128, HA], BF16,
                                  kind="Internal", addr_space="Shared")
            if HB:
                zAGb = dp.tile([16, HB], BF16, tag="zAGb")
                h1AGb = dp.tile([16, HB], BF16, tag="h1AGb")
                tb1b = nc.dram_tensor("tb1b", [128, HB], BF16,
                                      kind="Internal", addr_space="Shared")
                tb2b = nc.dram_tensor("tb2b", [128, HB], BF16,
                                      kind="Internal", addr_space="Shared")

            def ag(in_ap, out_ap):
                nc.gpsimd.collective_compute(
                    "AllGather", ALU.bypass,
                    replica_groups=[list(range(NC))],
                    ins=[in_ap], outs=[out_ap],
                )

            # ---- z'^T = dinv * (x @ W1)^T, in 512-col chunks ----
            dma_engines = [nc.sync, nc.scalar]
            for j in range(NDP // 512):
                zp = zpp.tile([16, 512], F32, tag="zp")
                for kc in range(KC):
                    xa = xtp.tile([128, 512], BF16, tag="xa")
                    dma_engines[(j * KC + kc) % 2].dma_start(
                        out=xa[:],
                        in_=xT_d[kc * 128:(kc + 1) * 128,
                                 j * 512:(j + 1) * 512])
                    nc.tensor.matmul(out=zp[:], lhsT=w1[kc][:], rhs=xa[:],
                                     start=(kc == 0), stop=(kc == KC - 1))
                dv = xtp.tile([16, 512], F32, tag="dv")
                nc.sync.dma_start(out=dv[:],
                                  in_=dinv_d[:, j * 512:(j + 1) * 512])
                zs = zsp.tile([16, 512], BF16, tag="zs")
                nc.vector.tensor_tensor(out=zs[:], in0=zp[:], in1=dv[:],
                                        op=ALU.mult)
                nc.sync.dma_start(out=zT_dram[:, j * 512:(j + 1) * 512],
                                  in_=zs[:])
                if HB and j == HA // 512 + 1:
                    nc.sync.dma_start(out=zAGa[:, :], in_=zT_dram[:, 0:HA])
                    ag(zAGa[:, :], tb1a[:, :])

            if HB:
                nc.sync.dma_start(out=zAGb[:, :], in_=zT_dram[:, HA:P])
                ag(zAGb[:, :], tb1b[:, :])
            else:
                nc.sync.dma_start(out=zAGa[:, :], in_=zT_dram[:, 0:P])
                ag(zAGa[:, :], tb1a[:, :])

            table = tbp.tile([128, P + 16], F32, tag="table")
            tableB = tbp.tile([128, P], BF16, tag="tableB")
            nc.vector.memset(table[:, P:P + 16], 0.0)
            partial = pp.tile([128, PW], F32, tag="partial")

            def layer(ta_dram, tb_dram, self_dram, is_last, hook=None):
                # spread the 3.2MB bf16 table load across engine DGE
                # queues, then widen to the fp32 gather table on DVE
                q = HA // 2
                nc.sync.dma_start(out=tableB[:, 0:q], in_=ta_dram[:, 0:q])
                nc.scalar.dma_start(out=tableB[:, q:HA],
                                    in_=ta_dram[:, q:HA])
                if HB:
                    q2 = HB // 2
                    nc.gpsimd.dma_start(out=tableB[:, HA:HA + q2],
                                        in_=tb_dram[:, 0:q2])
                    nc.sync.dma_start(out=tableB[:, HA + q2:P],
                                      in_=tb_dram[:, q2:HB])
                nc.vector.tensor_scalar(out=table[:, 0:P], in0=tableB[:],
                                        scalar1=1.0, scalar2=None,
                                        op0=ALU.mult)
                nc.vector.memset(partial[:], 0.0)
                for ch in range(NCH):
                    u = used[ch]
                    gt = gp.tile([128, NI], F32, tag="gt")
                    nc.gpsimd.ap_gather(
                        out_ap=gt[:, 0:u], in_ap=table[:],
                        idxs_ap=idxs[:, ch * (NI // 16):
                                     ch * (NI // 16) + u // 16],
                        channels=128, num_elems=P + 16, d=1, num_idxs=u,
                    )
                    for (col, k, nd, ppos) in sched[ch]:
                        nc.vector.tensor_reduce(
                            out=partial[:, ppos:ppos + nd],
                            in_=_ap(gt[:], col, [[k, nd], [1, k]]),
                            axis=mybir.AxisListType.X, op=ALU.add,
                        )
                # realign + combine + post, per 512-dst chunk
                RNI = 1024
                nrch = (NDP + RNI - 1) // RNI
                for rc in range(nrch):
                    if hook is not None and rc == 8:
                        hook()
                    w = min(RNI, NDP - rc * RNI)
                    ra = rap.tile([128, RNI], F32, tag="ra")
                    nc.gpsimd.ap_gather(
                        out_ap=ra[:, 0:w], in_ap=partial[:],
                        idxs_ap=rids[:, rc * (RNI // 16):
                                     rc * (RNI // 16) + w // 16],
                        channels=128, num_elems=PW, d=1, num_idxs=w,
                    )
                    for j in range(w // 512):
                        cols = slice(rc * RNI + j * 512,
                                     rc * RNI + j * 512 + 512)
                        ap_ = lpp.tile([16, 512], F32, tag="ap_")
                        nc.tensor.matmul(
                            out=ap_[:], lhsT=m16[:],
                            rhs=ra[:, j * 512:(j + 1) * 512],
                            start=True, stop=True)
                        sf = pop.tile([16, 512], BF16, tag="sf")
                        nc.sync.dma_start(out=sf[:], in_=self_dram[:, cols])
                        dv = pop.tile([16, 512], F32, tag="dv2")
                        nc.sync.dma_start(out=dv[:], in_=dinv_d[:, cols])
                        ag = pop.tile([16, 512], F32, tag="ag")
                        nc.vector.tensor_tensor(out=ag[:], in0=ap_[:],
                                                in1=sf[:], op=ALU.add)
                        nc.vector.tensor_tensor(out=ag[:], in0=ag[:],
                                                in1=dv[:], op=ALU.mult)
                        if not is_last:
                            nc.vector.tensor_tensor(
                                out=ag[:], in0=ag[:],
                                in1=_ap(b1c[:], 0, [[0, 512]]), op=ALU.add)
                            nc.scalar.activation(out=ag[:], in_=ag[:],
                                                 func=AF.Relu)
                            hb = pop.tile([16, 512], BF16, tag="hb")
                            nc.vector.tensor_tensor(out=hb[:], in0=ag[:],
                                                    in1=dv[:], op=ALU.mult)
                            nc.sync.dma_start(out=h1_dram[:, cols], in_=hb[:])
                        else:
                            # logits + log_softmax, 4x128 dsts batched
                            t2 = (rc * RNI + j * 512) // 128
                            lp = lpp.tile([128, 4 * C], F32, tag="lp")
                            for i in range(4):
                                nc.tensor.matmul(
                                    out=lp[:, i * C:(i + 1) * C],
                                    lhsT=ag[:, i * 128:(i + 1) * 128],
                                    rhs=w2r[:], start=True, stop=True)
                            lt = pop.tile([128, 4 * C], F32, tag="lt")
                            nc.vector.tensor_tensor(
                                out=lt[:], in0=lp[:],
                                in1=_ap(b2r[:], 0, [[0, 4], [1, C]]),
                                op=ALU.add)
                            nm = pop.tile([128, 4], F32, tag="nm")
                            nc.vector.tensor_reduce(
                                out=nm[:],
                                in_=_ap(lt[:], 0, [[C, 4], [1, C]]),
                                axis=mybir.AxisListType.X,
                                op=ALU.max, negate=True)
                            nc.vector.tensor_tensor(
                                out=lt[:], in0=lt[:],
                                in1=_ap(nm[:], 0, [[1, 4], [0, C]]),
                                op=ALU.add)
                            et = pop.tile([128, 4 * C], F32, tag="et")
                            nc.scalar.activation(out=et[:], in_=lt[:],
                                                 func=AF.Exp)
                            nc.vector.tensor_reduce(
                                out=nm[:],
                                in_=_ap(et[:], 0, [[C, 4], [1, C]]),
                                axis=mybir.AxisListType.X, op=ALU.add)
                            nc.scalar.activation(out=nm[:], in_=nm[:],
                                                 func=AF.Ln)
                            nc.vector.tensor_tensor(
                                out=lt[:], in0=lt[:],
                                in1=_ap(nm[:], 0, [[1, 4], [0, C]]),
                                op=ALU.subtract)
                            nc.sync.dma_start(
                                out=out_d[:, t2 * C:(t2 + 4) * C],
                                in_=lt[:])

            def l1_hook():
                if HB:
                    nc.sync.dma_start(out=h1AGa[:, :], in_=h1_dram[:, 0:HA])
                    ag(h1AGa[:, :], tb2a[:, :])

            layer(tb1a, tb1b if HB else None, zT_dram, is_last=False,
                  hook=l1_hook)
            if HB:
                nc.sync.dma_start(out=h1AGb[:, :], in_=h1_dram[:, HA:P])
                ag(h1AGb[:, :], tb2b[:, :])
            else:
                nc.sync.dma_start(out=h1AGa[:, :], in_=h1_dram[:, 0:P])
                ag(h1AGa[:, :], tb2a[:, :])
            layer(tb2a, tb2b if HB else None, h1_dram, is_last=True)

    return nc


# ---------------------------------------------------------------------------
# Entry point
# ---------------------------------------------------------------------------

def kernel(x, edge_index, W1, b1, W2, b2):
    N, F = x.shape
    HID = W1.shape[1]
    C = W2.shape[1]
    P = N // NC
    src = np.asarray(edge_index[0], dtype=np.int64)
    dst = np.asarray(edge_index[1], dtype=np.int64)

    deg = np.bincount(dst, minlength=N).astype(np.int64) + 1
    dinv = (1.0 / np.sqrt(deg.astype(np.float64))).astype(np.float32)

    plan = _make_plan(src, dst, N)
    NDP = plan["NDP"]
    nc = _build(N, F, HID, C, plan)

    x = np.asarray(x, dtype=np.float32)
    W1 = np.ascontiguousarray(
        np.asarray(W1, dtype=np.float32).astype(ml_dtypes.bfloat16))
    W2 = np.ascontiguousarray(np.asarray(W2, dtype=np.float32))
    b2r = np.tile(np.asarray(b2, dtype=np.float32)[None, :], (128, 1))
    M16 = np.zeros((128, HID), dtype=np.float32)
    for r in range(NC):
        M16[16 * r:16 * r + 16, :] = np.eye(HID, dtype=np.float32)

    in_maps = []
    for c in range(NC):
        xT = np.zeros((F, NDP), dtype=ml_dtypes.bfloat16)
        xT[:, :P] = x[c * P:(c + 1) * P].T.astype(ml_dtypes.bfloat16)
        d16 = np.zeros((16, NDP), dtype=np.float32)
        d16[:, :P] = dinv[c * P:(c + 1) * P][None, :]
        in_maps.append({
            "xT": np.ascontiguousarray(xT),
            "W1": W1,
            "b1c": np.ascontiguousarray(
                np.asarray(b1, np.float32).reshape(16, 1)),
            "W2r": W2,
            "b2r": b2r,
            "M16": M16,
            "dinv16": np.ascontiguousarray(d16),
            "idxs": np.ascontiguousarray(
                plan["idx_data"][c].transpose(1, 0, 2).reshape(128, -1)),
            "rids": np.ascontiguousarray(plan["rid_data"][c]),
        })

    trace = bool(int(os.environ.get("GCN_TRACE", "0")))
    if int(os.environ.get("GCN_SIM", "0")):
        from concourse.bass_interp import MultiCoreSim

        sim = MultiCoreSim(nc, num_cores=NC, trace=False)
        for c, core in enumerate(sim.cores.values()):
            for k, v in in_maps[c].items():
                core.tensor(k)[:] = v
        sim.simulate(check_with_hw=False)
        results = [{"out": np.array(core.tensor("out"))}
                   for core in sim.cores.values()]
        _last_result["exec_time_ns"] = None
    else:
        nc.finalize()
        br = bass_utils.run_bass_kernel_spmd(
            nc, in_maps, core_ids=list(range(NC)), trace=trace,
        )
        results = br.results
        _last_result["exec_time_ns"] = br.exec_time_ns

    _last_result["results"] = results
    _last_result["plan"] = plan

    out = np.empty((N, C), dtype=np.float32)
    for c in range(NC):
        arr = results[c]["out"].reshape(128, NDP // 128, C)
        arr = arr.transpose(1, 0, 2).reshape(NDP, C)
        out[c * P:(c + 1) * P] = arr[:P]
    return out



# revision 17
# speedup vs baseline: 1.1030x; 1.0066x over previous
"""GCN v3: ap_gather-based edge gather on 8 TRN2 cores.

Layout: nodes sharded 8 ways (core c owns dst range [cP,(c+1)P)).  Tables
live transposed in SBUF as [128 = 8 src-ranges x 16 feats, P nodes]; each
16-partition GPSIMD group gathers edges whose src falls in its range
(ap_gather, group-private int16 index lists).  Per (dst, range) segment
sums run on DVE (K-run reduces, K-desc order, SPMD-global structure);
partials are realigned to global dst order by a second ap_gather and
summed across ranges by one PE matmul.  Self-loop terms are added
directly from the core's own z'/h1' columns (no gather).  Both layers
share one index/schedule set since the graph is identical.

v4 over v2 (4.07ms -> 3.72ms measured, rel err 1.2e-4):
 - suffix-max slot schedule with zero-pad column: a class-k dst may sit in
   a K'>=k slot padded with gathers of a zeroed table column, so slot
   capacities cover suffix maxima over (core,range) pairs instead of
   per-class maxima (~53.5k -> ~50.8k gather columns per group per layer;
   ap_gather costs ~27ns per index on each of the 8 DSPs, which is the
   kernel's dominant cost).
 - log_softmax tail batched 4x128 dsts wide (one DVE/Act chain per 512).
 - AllGather outputs in Shared DRAM scratchpad (fast collective path).
 - table/x loads spread across sync/scalar/gpsimd DGE queues.
 - x/W1/z'/h1/AllGather/table traffic in bf16 (half the DMA bytes on the
   critical start/mid windows); tables widened to fp32 in SBUF after load
   since ap_gather fetches 4-byte columns.
 - AllGathers split in two halves: the first half is triggered while the
   producing phase (z chunks / layer-1 realign+post) is still running, so
   the collective and the next table load hide under GPSIMD work.
"""

import os
import sys

for _p in ("/opt/trn_rl_repo", "/opt/pypackages"):
    if _p not in sys.path:
        sys.path.insert(0, _p)

import ml_dtypes
import numpy as np

from concourse import bacc, bass, tile, mybir, library_config
from concourse import bass_utils

F32 = mybir.dt.float32
BF16 = mybir.dt.bfloat16
I16 = mybir.dt.int16
AF = mybir.ActivationFunctionType
ALU = mybir.AluOpType

NC = 8
NI = 4096          # gather columns per ap_gather call

_last_result = {}


# ---------------------------------------------------------------------------
# Host-side plan
# ---------------------------------------------------------------------------

def _make_plan(src, dst, N):
    P = N // NC
    ZC = P  # zero column index (table has 16 zeroed pad columns at P..P+15)
    core_d = dst // P
    rng_s = src // P
    dloc = dst - core_d * P
    sloc = src - rng_s * P

    # per (core, range): dst counts
    K_cr = []        # K_cr[c][r] = dict-like arrays: (dsts_sorted, counts)
    KMAX = 0
    for c in range(NC):
        row = []
        mc = core_d == c
        for r in range(NC):
            m = mc & (rng_s == r)
            d_ = dloc[m]
            s_ = sloc[m]
            cnt = np.bincount(d_, minlength=P)
            KMAX = max(KMAX, int(cnt.max()))
            row.append((d_, s_, cnt))
        K_cr.append(row)

    # ONE slot-structure shared by all (core, range) pairs so every reduce is
    # full-128-partition.  A class-k dst may occupy a K'>=k slot, padding the
    # run with gathers of the zero column, so capacities only need to cover
    # the suffix maxima (near-zero padding) instead of per-class maxima.
    sufmax = np.zeros(KMAX + 2, dtype=np.int64)
    for c in range(NC):
        for r in range(NC):
            cnt = K_cr[c][r][2]
            ks, nds = np.unique(cnt[cnt > 0], return_counts=True)
            cc = np.zeros(KMAX + 2, dtype=np.int64)
            cc[ks] = nds
            suf = cc[::-1].cumsum()[::-1]
            np.maximum(sufmax, suf, out=sufmax)
    nd_g = sufmax - np.append(sufmax[1:], 0)
    struct = [(k, int(nd_g[k])) for k in range(KMAX, 0, -1) if nd_g[k] > 0]

    # chunked schedule: entries (coloff, K, nd, ppos); runs never straddle
    # chunk boundaries; identical for every class/core.
    sched = []
    ch = 0
    col = 0
    ppos = 1
    for (k, nd) in struct:
        left = nd
        while left > 0:
            while ch >= len(sched):
                sched.append([])
            fit = min(left, (NI - col) // k)
            if fit == 0:
                ch += 1
                col = 0
                continue
            sched[ch].append((col, k, fit, ppos))
            col += fit * k
            ppos += fit
            left -= fit
    NCH = len(sched)
    SL = NCH * NI
    PW = ppos + 2 - (ppos % 2)  # even pad
    # per-chunk used columns (mult of 16): trim the gather of the tail chunk
    used_cols = []
    for ch_e in sched:
        u = max(col + k * nd for (col, k, nd, _) in ch_e)
        used_cols.append(min(NI, ((u + 15) // 16) * 16))

    # per-core idx streams + partial position of each (dst, r)
    idx_data = np.full((NC, NCH, 128, NI // 16), ZC, dtype=np.int16)
    pos_cr = np.full((NC, NC, P), 0, dtype=np.int32)  # [c][r][dst] -> ppos
    for c in range(NC):
        for r in range(NC):
            d_, s_, cnt = K_cr[c][r]
            order = np.lexsort((s_, d_))
            d_s = d_[order]
            s_s = s_[order]
            starts = np.searchsorted(d_s, np.arange(P))
            ends = np.searchsorted(d_s, np.arange(P), side="right")
            # all dsts with count>0, sorted by count desc (dst asc within)
            nz = np.where(cnt > 0)[0]
            queue = nz[np.argsort(-cnt[nz], kind="stable")]
            qi = 0
            stream = np.full(NCH * NI, ZC, dtype=np.int16)
            # walk the same schedule the device uses (slots in K-desc order)
            for ch in range(NCH):
                base = ch * NI
                for (col, k, fit, ppos) in sched[ch]:
                    take = queue[qi:qi + fit]
                    qi += len(take)
                    for j, dd in enumerate(take):
                        kk = ends[dd] - starts[dd]
                        assert kk <= k, (kk, k)
                        sl = s_s[starts[dd]:ends[dd]]
                        o = base + col + j * k
                        stream[o:o + kk] = sl
                        pos_cr[c, r, dd] = ppos + j
            assert qi == len(queue), (qi, len(queue))
            # wrap into tiles: position i -> [16r + i%16, i//16]
            sw = stream.reshape(NCH, NI // 16, 16)
            idx_data[c, :, 16 * r:16 * r + 16, :] = sw.transpose(0, 2, 1)

    # realign indices: rid[c][r][j] = pos_cr or 0, j in [0, 12800)
    NDP = ((P + 511) // 512) * 512  # padded dst cols (512-mult)
    rid_data = np.zeros((NC, 128, NDP // 16), dtype=np.int16)
    for c in range(NC):
        for r in range(NC):
            v = np.zeros(NDP, dtype=np.int16)
            v[:P] = pos_cr[c, r].astype(np.int16)
            rid_data[c, 16 * r:16 * r + 16, :] = v.reshape(NDP // 16, 16).T
    return dict(P=P, SL=SL, NCH=NCH, PW=PW, NDP=NDP, sched=sched,
                used=used_cols, idx_data=idx_data, rid_data=rid_data)


# ---------------------------------------------------------------------------
# Device program
# ---------------------------------------------------------------------------

def _ap(t_ap, offset, dims):
    return bass.AP(t_ap.tensor, t_ap.offset + offset, [list(t_ap.ap[0])] + dims)


def _build(N, F, HID, C, plan):
    P = plan["P"]
    NCH = plan["NCH"]
    PW = plan["PW"]
    NDP = plan["NDP"]
    sched = plan["sched"]
    used = plan["used"]
    NT2 = NDP // 128          # logits tiles
    KC = F // 128

    nc = bacc.Bacc(None, target_bir_lowering=False, debug=False,
                   num_devices=NC)

    xT_d = nc.dram_tensor("xT", [F, NDP], BF16, kind="ExternalInput")
    w1_d = nc.dram_tensor("W1", [F, HID], BF16, kind="ExternalInput")
    b1_d = nc.dram_tensor("b1c", [16, 1], F32, kind="ExternalInput")
    w2_d = nc.dram_tensor("W2r", [HID, C], F32, kind="ExternalInput")
    b2_d = nc.dram_tensor("b2r", [128, C], F32, kind="ExternalInput")
    m16_d = nc.dram_tensor("M16", [128, HID], F32, kind="ExternalInput")
    dinv_d = nc.dram_tensor("dinv16", [16, NDP], F32, kind="ExternalInput")
    idx_d = nc.dram_tensor("idxs", [128, NCH * (NI // 16)], I16,
                           kind="ExternalInput")
    rid_d = nc.dram_tensor("rids", [128, NDP // 16], I16,
                           kind="ExternalInput")
    out_d = nc.dram_tensor("out", [128, NT2 * C], F32, kind="ExternalOutput")

    with tile.TileContext(nc) as tc:
        with (
            tc.tile_pool(name="const", bufs=1) as cp,
            tc.tile_pool(name="dram", bufs=1, space="DRAM") as dp,
            tc.tile_pool(name="xt", bufs=3) as xtp,
            tc.tile_pool(name="zp", bufs=2, space="PSUM") as zpp,
            tc.tile_pool(name="zs", bufs=2) as zsp,
            tc.tile_pool(name="tab", bufs=1) as tbp,
            tc.tile_pool(name="idx", bufs=1) as ixp,
            tc.tile_pool(name="g", bufs=2) as gp,
            tc.tile_pool(name="part", bufs=1) as pp,
            tc.tile_pool(name="ra", bufs=3) as rap,
            tc.tile_pool(name="post", bufs=3) as pop,
            tc.tile_pool(name="lp", bufs=2, space="PSUM") as lpp,
        ):
            nc.gpsimd.load_library(library_config.ap_gather)

            w1 = []
            for kc in range(KC):
                t = cp.tile([128, HID], BF16, tag=f"w1_{kc}")
                nc.sync.dma_start(out=t[:],
                                  in_=w1_d[kc * 128:(kc + 1) * 128, :])
                w1.append(t)
            b1c = cp.tile([16, 1], F32, tag="b1c")
            nc.sync.dma_start(out=b1c[:], in_=b1_d[:, :])
            w2r = cp.tile([HID, C], F32, tag="w2r")
            nc.sync.dma_start(out=w2r[:], in_=w2_d[:, :])
            b2r = cp.tile([128, C], F32, tag="b2r")
            nc.sync.dma_start(out=b2r[:], in_=b2_d[:, :])
            m16 = cp.tile([128, HID], F32, tag="m16")
            nc.sync.dma_start(out=m16[:], in_=m16_d[:, :])
            idxs = cp.tile([128, NCH * (NI // 16)], I16, tag="idxs")
            nc.sync.dma_start(out=idxs[:], in_=idx_d[:, :])
            rids = cp.tile([128, NDP // 16], I16, tag="rids")
            nc.sync.dma_start(out=rids[:], in_=rid_d[:, :])

            zT_dram = dp.tile([16, NDP], BF16, tag="zT")
            h1_dram = dp.tile([16, NDP], BF16, tag="h1T")
            # half-split collectives: first half AllGathers while the tail
            # of the producing phase still runs
            HA = (P // 1024) * 512 if P >= 4096 else P
            HB = P - HA
            zAGa = dp.tile([16, HA], BF16, tag="zAGa")
            h1AGa = dp.tile([16, HA], BF16, tag="h1AGa")
            tb1a = nc.dram_tensor("tb1a", [128, HA], BF16,
                                  kind="Internal", addr_space="Shared")
            tb2a = nc.dram_tensor("tb2a", [128, HA], BF16,
                                  kind="Internal", addr_space="Shared")
            if HB:
                zAGb = dp.tile([16, HB], BF16, tag="zAGb")
                h1AGb = dp.tile([16, HB], BF16, tag="h1AGb")
                tb1b = nc.dram_tensor("tb1b", [128, HB], BF16,
                                      kind="Internal", addr_space="Shared")
                tb2b = nc.dram_tensor("tb2b", [128, HB], BF16,
                                      kind="Internal", addr_space="Shared")

            def ag(in_ap, out_ap):
                nc.gpsimd.collective_compute(
                    "AllGather", ALU.bypass,
                    replica_groups=[list(range(NC))],
                    ins=[in_ap], outs=[out_ap],
                )

            # ---- z'^T = dinv * (x @ W1)^T, in 512-col chunks ----
            dma_engines = [nc.sync, nc.scalar]
            for j in range(NDP // 512):
                zp = zpp.tile([16, 512], F32, tag="zp")
                for kc in range(KC):
                    xa = xtp.tile([128, 512], BF16, tag="xa")
                    dma_engines[(j * KC + kc) % 2].dma_start(
                        out=xa[:],
                        in_=xT_d[kc * 128:(kc + 1) * 128,
                                 j * 512:(j + 1) * 512])
                    nc.tensor.matmul(out=zp[:], lhsT=w1[kc][:], rhs=xa[:],
                                     start=(kc == 0), stop=(kc == KC - 1))
                dv = xtp.tile([16, 512], F32, tag="dv")
                nc.sync.dma_start(out=dv[:],
                                  in_=dinv_d[:, j * 512:(j + 1) * 512])
                zs = zsp.tile([16, 512], BF16, tag="zs")
                nc.vector.tensor_tensor(out=zs[:], in0=zp[:], in1=dv[:],
                                        op=ALU.mult)
                nc.sync.dma_start(out=zT_dram[:, j * 512:(j + 1) * 512],
                                  in_=zs[:])
                if HB and j == HA // 512 + 1:
                    nc.sync.dma_start(out=zAGa[:, :], in_=zT_dram[:, 0:HA])
                    ag(zAGa[:, :], tb1a[:, :])

            if HB:
                nc.sync.dma_start(out=zAGb[:, :], in_=zT_dram[:, HA:P])
                ag(zAGb[:, :], tb1b[:, :])
            else:
                nc.sync.dma_start(out=zAGa[:, :], in_=zT_dram[:, 0:P])
                ag(zAGa[:, :], tb1a[:, :])

            table = tbp.tile([128, P + 16], F32, tag="table")
            tableB = tbp.tile([128, P], BF16, tag="tableB")
            nc.vector.memset(table[:, P:P + 16], 0.0)
            partial = pp.tile([128, PW], F32, tag="partial")

            def load_a_dmas(src):
                # bf16 half-loads; issued only after their collective has
                # completed so no engine queue stalls on the sem wait
                q = HA // 2
                nc.sync.dma_start(out=tableB[:, 0:q], in_=src[:, 0:q])
                nc.scalar.dma_start(out=tableB[:, q:HA], in_=src[:, q:HA])

            def load_b_dmas(src):
                q2 = HB // 2
                nc.scalar.dma_start(out=tableB[:, HA:HA + q2],
                                    in_=src[:, 0:q2])
                nc.sync.dma_start(out=tableB[:, HA + q2:P],
                                  in_=src[:, q2:HB])

            def cast_table():
                nc.vector.tensor_scalar(out=table[:, 0:P], in0=tableB[:],
                                        scalar1=1.0, scalar2=None,
                                        op0=ALU.mult)

            def layer(self_dram, is_last, hook=None):
                cast_table()
                nc.vector.memset(partial[:], 0.0)
                for ch in range(NCH):
                    u = used[ch]
                    gt = gp.tile([128, NI], F32, tag="gt")
                    nc.gpsimd.ap_gather(
                        out_ap=gt[:, 0:u], in_ap=table[:],
                        idxs_ap=idxs[:, ch * (NI // 16):
                                     ch * (NI // 16) + u // 16],
                        channels=128, num_elems=P + 16, d=1, num_idxs=u,
                    )
                    for (col, k, nd, ppos) in sched[ch]:
                        nc.vector.tensor_reduce(
                            out=partial[:, ppos:ppos + nd],
                            in_=_ap(gt[:], col, [[k, nd], [1, k]]),
                            axis=mybir.AxisListType.X, op=ALU.add,
                        )
                # realign + combine + post, per 512-dst chunk
                RNI = 1024
                nrch = (NDP + RNI - 1) // RNI
                for rc in range(nrch):
                    if hook is not None and rc in (8, 11):
                        hook(rc)
                    w = min(RNI, NDP - rc * RNI)
                    ra = rap.tile([128, RNI], F32, tag="ra")
                    nc.gpsimd.ap_gather(
                        out_ap=ra[:, 0:w], in_ap=partial[:],
                        idxs_ap=rids[:, rc * (RNI // 16):
                                     rc * (RNI // 16) + w // 16],
                        channels=128, num_elems=PW, d=1, num_idxs=w,
                    )
                    for j in range(w // 512):
                        cols = slice(rc * RNI + j * 512,
                                     rc * RNI + j * 512 + 512)
                        ap_ = lpp.tile([16, 512], F32, tag="ap_")
                        nc.tensor.matmul(
                            out=ap_[:], lhsT=m16[:],
                            rhs=ra[:, j * 512:(j + 1) * 512],
                            start=True, stop=True)
                        sf = pop.tile([16, 512], BF16, tag="sf")
                        nc.sync.dma_start(out=sf[:], in_=self_dram[:, cols])
                        dv = pop.tile([16, 512], F32, tag="dv2")
                        nc.sync.dma_start(out=dv[:], in_=dinv_d[:, cols])
                        ag = pop.tile([16, 512], F32, tag="ag")
                        nc.vector.tensor_tensor(out=ag[:], in0=ap_[:],
                                                in1=sf[:], op=ALU.add)
                        nc.vector.tensor_tensor(out=ag[:], in0=ag[:],
                                                in1=dv[:], op=ALU.mult)
                        if not is_last:
                            nc.vector.tensor_tensor(
                                out=ag[:], in0=ag[:],
                                in1=_ap(b1c[:], 0, [[0, 512]]), op=ALU.add)
                            nc.scalar.activation(out=ag[:], in_=ag[:],
                                                 func=AF.Relu)
                            hb = pop.tile([16, 512], BF16, tag="hb")
                            nc.vector.tensor_tensor(out=hb[:], in0=ag[:],
                                                    in1=dv[:], op=ALU.mult)
                            nc.sync.dma_start(out=h1_dram[:, cols], in_=hb[:])
                        else:
                            # logits + log_softmax, 4x128 dsts batched
                            t2 = (rc * RNI + j * 512) // 128
                            lp = lpp.tile([128, 4 * C], F32, tag="lp")
                            for i in range(4):
                                nc.tensor.matmul(
                                    out=lp[:, i * C:(i + 1) * C],
                                    lhsT=ag[:, i * 128:(i + 1) * 128],
                                    rhs=w2r[:], start=True, stop=True)
                            lt = pop.tile([128, 4 * C], F32, tag="lt")
                            nc.vector.tensor_tensor(
                                out=lt[:], in0=lp[:],
                                in1=_ap(b2r[:], 0, [[0, 4], [1, C]]),
                                op=ALU.add)
                            nm = pop.tile([128, 4], F32, tag="nm")
                            nc.vector.tensor_reduce(
                                out=nm[:],
                                in_=_ap(lt[:], 0, [[C, 4], [1, C]]),
                                axis=mybir.AxisListType.X,
                                op=ALU.max, negate=True)
                            nc.vector.tensor_tensor(
                                out=lt[:], in0=lt[:],
                                in1=_ap(nm[:], 0, [[1, 4], [0, C]]),
                                op=ALU.add)
                            et = pop.tile([128, 4 * C], F32, tag="et")
                            nc.scalar.activation(out=et[:], in_=lt[:],
                                                 func=AF.Exp)
                            nc.vector.tensor_reduce(
                                out=nm[:],
                                in_=_ap(et[:], 0, [[C, 4], [1, C]]),
                                axis=mybir.AxisListType.X, op=ALU.add)
                            nc.scalar.activation(out=nm[:], in_=nm[:],
                                                 func=AF.Ln)
                            nc.vector.tensor_tensor(
                                out=lt[:], in0=lt[:],
                                in1=_ap(nm[:], 0, [[1, 4], [0, C]]),
                                op=ALU.subtract)
                            nc.sync.dma_start(
                                out=out_d[:, t2 * C:(t2 + 4) * C],
                                in_=lt[:])

            def l1_hook(rc):
                if not HB:
                    return
                if rc == 8:
                    nc.sync.dma_start(out=h1AGa[:, :], in_=h1_dram[:, 0:HA])
                    ag(h1AGa[:, :], tb2a[:, :])
                elif rc == 11:
                    load_a_dmas(tb2a)

            load_a_dmas(tb1a)
            if HB:
                load_b_dmas(tb1b)
            layer(zT_dram, is_last=False, hook=l1_hook)
            if HB:
                nc.sync.dma_start(out=h1AGb[:, :], in_=h1_dram[:, HA:P])
                ag(h1AGb[:, :], tb2b[:, :])
                load_b_dmas(tb2b)
            else:
                nc.sync.dma_start(out=h1AGa[:, :], in_=h1_dram[:, 0:P])
                ag(h1AGa[:, :], tb2a[:, :])
                load_a_dmas(tb2a)
            layer(h1_dram, is_last=True)

    return nc


# ---------------------------------------------------------------------------
# Entry point
# ---------------------------------------------------------------------------

def kernel(x, edge_index, W1, b1, W2, b2):
    N, F = x.shape
    HID = W1.shape[1]
    C = W2.shape[1]
    P = N // NC
    src = np.asarray(edge_index[0], dtype=np.int64)
    dst = np.asarray(edge_index[1], dtype=np.int64)

    deg = np.bincount(dst, minlength=N).astype(np.int64) + 1
    dinv = (1.0 / np.sqrt(deg.astype(np.float64))).astype(np.float32)

    plan = _make_plan(src, dst, N)
    NDP = plan["NDP"]
    nc = _build(N, F, HID, C, plan)

    x = np.asarray(x, dtype=np.float32)
    W1 = np.ascontiguousarray(
        np.asarray(W1, dtype=np.float32).astype(ml_dtypes.bfloat16))
    W2 = np.ascontiguousarray(np.asarray(W2, dtype=np.float32))
    b2r = np.tile(np.asarray(b2, dtype=np.float32)[None, :], (128, 1))
    M16 = np.zeros((128, HID), dtype=np.float32)
    for r in range(NC):
        M16[16 * r:16 * r + 16, :] = np.eye(HID, dtype=np.float32)

    in_maps = []
    for c in range(NC):
        xT = np.zeros((F, NDP), dtype=ml_dtypes.bfloat16)
        xT[:, :P] = x[c * P:(c + 1) * P].T.astype(ml_dtypes.bfloat16)
        d16 = np.zeros((16, NDP), dtype=np.float32)
        d16[:, :P] = dinv[c * P:(c + 1) * P][None, :]
        in_maps.append({
            "xT": np.ascontiguousarray(xT),
            "W1": W1,
            "b1c": np.ascontiguousarray(
                np.asarray(b1, np.float32).reshape(16, 1)),
            "W2r": W2,
            "b2r": b2r,
            "M16": M16,
            "dinv16": np.ascontiguousarray(d16),
            "idxs": np.ascontiguousarray(
                plan["idx_data"][c].transpose(1, 0, 2).reshape(128, -1)),
            "rids": np.ascontiguousarray(plan["rid_data"][c]),
        })

    trace = bool(int(os.environ.get("GCN_TRACE", "0")))
    if int(os.environ.get("GCN_SIM", "0")):
        from concourse.bass_interp import MultiCoreSim

        sim = MultiCoreSim(nc, num_cores=NC, trace=False)
        for c, core in enumerate(sim.cores.values()):
            for k, v in in_maps[c].items():
                core.tensor(k)[:] = v
        sim.simulate(check_with_hw=False)
        results = [{"out": np.array(core.tensor("out"))}
                   for core in sim.cores.values()]
        _last_result["exec_time_ns"] = None
    else:
        nc.finalize()
        br = bass_utils.run_bass_kernel_spmd(
            nc, in_maps, core_ids=list(range(NC)), trace=trace,
        )
        results = br.results
        _last_result["exec_time_ns"] = br.exec_time_ns

    _last_result["results"] = results
    _last_result["plan"] = plan

    out = np.empty((N, C), dtype=np.float32)
    for c in range(NC):
        arr = results[c]["out"].reshape(128, NDP // 128, C)
        arr = arr.transpose(1, 0, 2).reshape(NDP, C)
        out[c * P:(c + 1) * P] = arr[:P]
    return out

